# revision 26
# baseline (speedup 1.0000x reference)
"""Trainium2 Bass kernel for nn_MultiHeadAttention (B=2, S=2048, D=2048, H=16).

Sharding: tensor-parallel over heads -- each of the 8 cores owns 2 heads
(both batches) for the q/k/v projections and attention, then two 8-way
AllToAlls (one per local head) convert the head-sharded attention output
Y^T into a token-sharded layout, and each core computes a disjoint
512-token slice of the output projection (no all-reduce needed).

Key structure (evolved against neuron-profile traces):
- All matmul operands are bf16; psum accumulation stays f32.  The PE
  sustains ~0.515 ns/col in collective-bearing programs (1.94 GHz;
  CC-free microbenches run at 2.4 GHz -- collectives throttle the PE
  ~18% program-wide, and the SWDGE remote_dma path that would avoid
  them crashes this runtime).  ~710k matmul cols/core => ~366 us PE
  floor; everything else hides behind it.
- q^T / k^T / v live in SBUF between phases; host pre-blocks all DRAM
  inputs into the exact [partition][...] layouts (contiguous DMAs).
- Projection streams x once (x tiles DMA'd BEFORE the 6 MB of weights
  so they are not starved on HBM bandwidth); 8 interleaved psum chains;
  V chains are double-buffered and run VLAG dc ahead so slice
  boundaries never wait on the q/k psum drains.
- Attention epilogue balance (per head-batch: PE 25us, DVE ~24, ACT 17):
  exp groups accumulate pairwise into a [P,2,IT] bf16 racc2 on the DVE
  (groups 0+1 combine in one op; the last group contracts directly on
  the PE, riding the AV pipeline); the two diagonal groups share one
  [P,4,IT] tile so a single batched DVE op applies the causal mask; AV
  emission lags two groups so the PE never waits on the exp/mask chain.
  V-bias and output bias fold into bo_eff = bo + wo@bv on the host
  (softmax rows sum to 1).
- A tiny warmup AllToAll during the projections absorbs the ~11 us CC
  arming cost so AllToAll(0) starts ~1 us after its trigger.
- Phase order: proj -> att(lh0) -> AllToAll(0) -> att(lh1) -> ya0
  readback -> AllToAll(1) -> ya1 readback -> out-proj.  att(lh1) covers
  AllToAll(0); out-proj pass 1 (all lh0 matmuls, partials staged in
  SBUF) covers AllToAll(1).  ya_dma(0) is emitted AFTER att(lh1)'s
  a2a_in writes so the in-order Sync queue never blocks them (the
  baseline lost 23 us + an out-proj DVFS ramp to that).
- Out-proj interleaves its two psum sub-chains (LDWEIGHTS stays hidden)
  and stages the output in bf16 (host upcasts) to halve the final DMA.
"""

import os
import sys

import numpy as np

_REPO = "/opt/trn_rl_repo"
if _REPO not in sys.path:
    sys.path.insert(0, _REPO)

from concourse import bacc, mybir, tile  # noqa: E402
import concourse.bass as bass  # noqa: E402

B, S, D, H = 2, 2048, 2048, 16
DH = D // H  # 128
NCORES = 8
HPC = H // NCORES  # heads per core = 2
JW = HPC * DH  # per-core head-feature width = 256
T = B * S  # 4096 flattened tokens
TSL = T // NCORES  # per-core output token slice = 512
SCALE = float(np.sqrt(DH))

F32 = mybir.dt.float32
BF16 = mybir.dt.bfloat16
AF = mybir.ActivationFunctionType
ALU = bass.mybir.AluOpType

P = 128
IT = 512  # query i-tile width
NIT = S // IT  # 4 i-tiles per (batch, head)
NJC = S // P  # 16 key chunks per batch
NDC = D // P  # 16 contraction chunks
NTS = T // IT  # 8 token slices (batch 0 first, then batch 1)
XSUB = 8  # x dc-chunks per sub-tile (few big DMAs win)
NXS = NDC // XSUB  # 2 x sub-tiles per slice
VLAG = 6  # V chains run this many dc ahead (slice-boundary warmup)


def build_program():
    nc = bacc.Bacc(
        "TRN2",
        target_bir_lowering=False,
        debug=False,
        num_devices=NCORES,
    )

    # ---- kernel I/O (host pre-blocked; per-core values via in_maps) ----
    xb = nc.dram_tensor("xb", [NTS, P, NDC, IT], BF16, kind="ExternalInput").ap()
    wqb = nc.dram_tensor("wqb", [P, NDC, JW], BF16, kind="ExternalInput").ap()
    wkb = nc.dram_tensor("wkb", [P, NDC, JW], BF16, kind="ExternalInput").ap()
    wvb = nc.dram_tensor("wvb", [P, NDC, JW], BF16, kind="ExternalInput").ap()
    # wo split by key-chunk parity: even chunks feed lh=0, odd feed lh=1
    woE = nc.dram_tensor("woE", [P, NJC // 2, D], BF16, kind="ExternalInput").ap()
    woO = nc.dram_tensor("woO", [P, NJC // 2, D], BF16, kind="ExternalInput").ap()
    bqb = nc.dram_tensor("bqb", [P, HPC], F32, kind="ExternalInput").ap()
    bkb = nc.dram_tensor("bkb", [P, HPC], F32, kind="ExternalInput").ap()
    bob = nc.dram_tensor("bob", [P, NDC], F32, kind="ExternalInput").ap()
    # 4 diagonal-band mask patterns (1.0 = attend), [p][m][i]
    maskb = nc.dram_tensor("maskb", [P, 4, IT], BF16, kind="ExternalInput").ap()
    onesb = nc.dram_tensor("onesb", [P, P], BF16, kind="ExternalInput").ap()
    out = nc.dram_tensor("out", [P, NDC, TSL], BF16, kind="ExternalOutput").ap()

    with tile.TileContext(nc) as tc:
        with (
            tc.tile_pool(name="dram", bufs=1, space="DRAM") as dram,
            tc.tile_pool(name="const", bufs=1) as cpool,
            tc.tile_pool(name="persist", bufs=1) as ppool,
            tc.tile_pool(name="small", bufs=2) as small,
            tc.tile_pool(name="epool", bufs=2) as epool,
        ):
            # ---- persistent SBUF tiles ----
            qT_sb = {}
            kT_sb = {}
            for lh in range(HPC):
                for b in range(B):
                    qT_sb[(lh, b)] = ppool.tile([P, S], BF16, name=f"qT_{lh}_{b}")
                    kT_sb[(lh, b)] = ppool.tile([P, S], BF16, name=f"kT_{lh}_{b}")
            v_sb = {
                b: ppool.tile([P, NJC, JW], BF16, name=f"v_{b}") for b in range(B)
            }
            ya_sb = {
                lh: ppool.tile([P, NCORES, TSL], BF16, name=f"ya{lh}")
                for lh in range(HPC)
            }
            wo_sb = {
                0: ppool.tile([P, NJC // 2, D], BF16, name="woE"),
                1: ppool.tile([P, NJC // 2, D], BF16, name="woO"),
            }

            # per-local-head AllToAll buffers (blocks = dest core's i-slice)
            a2a_in = {
                lh: dram.tile([NCORES, DH, TSL], BF16, name=f"a2a_in_{lh}")
                for lh in range(HPC)
            }
            a2a_out = {
                lh: dram.tile([NCORES * DH, TSL], BF16, name=f"a2a_out_{lh}")
                for lh in range(HPC)
            }

            # tiny warmup AllToAll: absorbs the ~11 us CC arming cost
            # while the projections run, so the real AllToAll(0) starts fast
            a2aw_in = dram.tile([NCORES, P, 4], BF16, name="a2aw_in")
            a2aw_out = dram.tile([NCORES * P, 4], BF16, name="a2aw_out")

            # ---- constants / weights ----
            wq_w = cpool.tile([P, NDC, JW], BF16)
            wk_w = cpool.tile([P, NDC, JW], BF16)
            wv_w = cpool.tile([P, NDC, JW], BF16)
            bq_sb = cpool.tile([P, HPC], F32)
            bk_sb = cpool.tile([P, HPC], F32)
            bo_sb = cpool.tile([P, NDC], F32)
            mask_sb = cpool.tile([P, 4, IT], BF16)
            ones_sb = cpool.tile([P, P], BF16)

            # warmup AllToAll emission: DVE memset + gpsimd-queue DMAs so
            # the Sync queue (x/weight streaming) is untouched
            wsrc = cpool.tile([P, 4], BF16, name="wsrc")
            nc.vector.memset(wsrc[:], 1.0)
            for gq in range(NCORES):
                nc.gpsimd.dma_start(a2aw_in[gq, :, :], wsrc[:])
            nc.gpsimd.collective_compute(
                "AllToAll",
                ALU.bypass,
                replica_groups=[list(range(NCORES))],
                ins=[a2aw_in[:].opt()],
                outs=[a2aw_out[:].opt()],
            )

            # ---------- projections (SBUF-resident outputs) ----------
            def proj_pass(tag):
                """q/k projections for both heads + V, streamed over the
                8 token slices.  V chains (bufs=2) run VLAG dc ahead so the
                slice boundary never waits on the q/k psum drains."""
                with (
                    tc.tile_pool(name=f"xpool{tag}", bufs=1) as xpool,
                    tc.tile_pool(name=f"psum_{tag}", bufs=1, space="PSUM") as psp,
                ):
                    for ts in range(NTS):
                        b, lt0 = ts // NIT, (ts % NIT) * IT
                        xs = []
                        for g in range(NXS):
                            xg = xpool.tile(
                                [P, XSUB, IT],
                                BF16,
                                tag="x",
                                bufs=2,
                                name=f"x{tag}_{ts}_{g}",
                            )
                            if ts == 0 and g == 0:
                                nc.sync.dma_start(
                                    xg[:, 0:2, :], xb[ts, :, 0:2, :]
                                )
                                nc.sync.dma_start(
                                    xg[:, 2:XSUB, :], xb[ts, :, 2:XSUB, :]
                                )
                            else:
                                nc.sync.dma_start(
                                    xg[:], xb[ts, :, g * XSUB : (g + 1) * XSUB, :]
                                )
                            xs.append(xg)
                        if ts == 0:
                            # weights AFTER ts0's x tiles so x isn't queued
                            # behind 6 MB on the shared HBM bandwidth
                            nc.sync.dma_start(wv_w[:, 0:4, :], wvb[:, 0:4, :])
                            nc.sync.dma_start(wk_w[:, 0:2, :], wkb[:, 0:2, :])
                            nc.sync.dma_start(wq_w[:, 0:2, :], wqb[:, 0:2, :])
                            nc.sync.dma_start(wv_w[:, 4:8, :], wvb[:, 4:8, :])
                            nc.sync.dma_start(wk_w[:, 2:5, :], wkb[:, 2:5, :])
                            nc.sync.dma_start(wq_w[:, 2:5, :], wqb[:, 2:5, :])
                            nc.sync.dma_start(wk_w[:, 5:8, :], wkb[:, 5:8, :])
                            nc.sync.dma_start(wq_w[:, 5:8, :], wqb[:, 5:8, :])
                            nc.sync.dma_start(wk_w[:, 8:12, :], wkb[:, 8:12, :])
                            nc.sync.dma_start(wq_w[:, 8:12, :], wqb[:, 8:12, :])
                            nc.sync.dma_start(wk_w[:, 12:16, :], wkb[:, 12:16, :])
                            nc.sync.dma_start(wq_w[:, 12:16, :], wqb[:, 12:16, :])
                            nc.sync.dma_start(wv_w[:, 8:16, :], wvb[:, 8:16, :])
                            nc.sync.dma_start(bk_sb[:], bkb)
                            nc.sync.dma_start(bq_sb[:], bqb)
                            nc.sync.dma_start(bo_sb[:], bob)
                            nc.sync.dma_start(mask_sb[:], maskb)
                            nc.sync.dma_start(ones_sb[:], onesb)
                        if ts == 2:
                            nc.sync.dma_start(wo_sb[0][:], woE)
                        if ts == 4:
                            nc.sync.dma_start(wo_sb[1][:], woO)

                        def xchunk(dc):
                            return xs[dc // XSUB][:, dc % XSUB, :]

                        pqk = {
                            nm: psp.tile(
                                [P, IT], F32, tag=nm, name=f"p{nm}_{ts}"
                            )
                            for nm in ("k0", "k1", "q0", "q1")
                        }
                        pv = {
                            half: psp.tile(
                                [P, 2, JW],
                                F32,
                                tag=f"v{half}",
                                bufs=2,
                                name=f"pv{ts}{half}",
                            )
                            for half in range(2)
                        }

                        def emit_v(dc):
                            sp = dc == NDC - 1
                            for tc2 in range(IT // P):
                                nc.tensor.matmul(
                                    pv[tc2 // 2][:, tc2 % 2, :],
                                    lhsT=xchunk(dc)[:, tc2 * P : (tc2 + 1) * P],
                                    rhs=wv_w[:, dc, :],
                                    start=(dc == 0 and tc2 % 2 == 0),
                                    stop=sp,
                                    skip_group_check=True,
                                )

                        # V warmup: first VLAG dc of the V chains keep the
                        # PE busy while this slice's q/k psum banks drain
                        for dc in range(VLAG):
                            emit_v(dc)
                        for dc in range(NDC):
                            st, sp = dc == 0, dc == NDC - 1
                            for h in range(HPC):
                                nc.tensor.matmul(
                                    pqk[f"k{h}"][:],
                                    lhsT=wk_w[:, dc, h * DH : (h + 1) * DH],
                                    rhs=xchunk(dc),
                                    start=st,
                                    stop=sp,
                                )
                                nc.tensor.matmul(
                                    pqk[f"q{h}"][:],
                                    lhsT=wq_w[:, dc, h * DH : (h + 1) * DH],
                                    rhs=xchunk(dc),
                                    start=st,
                                    stop=sp,
                                )
                            if dc < NDC - VLAG:
                                emit_v(dc + VLAG)
                        # epilogues on DVE: bias add, write bf16 persistents
                        for h in range(HPC):
                            nc.vector.tensor_tensor(
                                kT_sb[(h, b)][:, lt0 : lt0 + IT],
                                pqk[f"k{h}"][:],
                                bk_sb[:, h : h + 1].to_broadcast([P, IT]),
                                ALU.add,
                            )
                            nc.vector.tensor_tensor(
                                qT_sb[(h, b)][:, lt0 : lt0 + IT],
                                pqk[f"q{h}"][:],
                                bq_sb[:, h : h + 1].to_broadcast([P, IT]),
                                ALU.add,
                            )
                        for half in range(2):
                            # v bias folded into bo on the host
                            nc.vector.tensor_copy(
                                v_sb[b][
                                    :,
                                    lt0 // P + 2 * half : lt0 // P + 2 * half + 2,
                                    :,
                                ],
                                pv[half][:],
                            )

            # ---------- attention for one local head + its AllToAll ----------
            def attention(lh, pre_cc=None):
                with (
                    tc.tile_pool(name=f"psS{lh}", bufs=2, space="PSUM") as psS,
                    tc.tile_pool(name=f"psO{lh}", bufs=2, space="PSUM") as psO,
                    tc.tile_pool(name=f"psR{lh}", bufs=2, space="PSUM") as psR,
                ):
                    # softmax denominators: exp groups accumulate pairwise
                    # into racc2 [P,2,IT] on the DVE (one op per group),
                    # contracted by TWO ones-matmuls per i-tile.  That
                    # contraction + epilogue are emitted after the next
                    # tile's first scores group so the PE never waits on
                    # the DVE tail.
                    pending = None

                    def flush_pending():
                        nonlocal pending
                        if pending is None:
                            return
                        racc2, po, pr, b, it, had_direct = pending
                        pending = None
                        nc.tensor.matmul(
                            pr[:],
                            lhsT=ones_sb[:],
                            rhs=racc2[:, 0, :],
                            start=not had_direct,
                            stop=False,
                        )
                        nc.tensor.matmul(
                            pr[:],
                            lhsT=ones_sb[:],
                            rhs=racc2[:, 1, :],
                            start=False,
                            stop=True,
                        )
                        rinv = small.tile(
                            [P, IT], F32, tag="rinv", name=f"ri{lh}{b}{it}"
                        )
                        nc.vector.reciprocal_approx_fast(rinv[:], pr[:])
                        # v-bias and output bias are folded into bo on the
                        # host (softmax rows sum to 1), so y is just po/r
                        y_sb = small.tile(
                            [P, IT], BF16, tag="y", bufs=8, name=f"y{lh}{b}{it}"
                        )
                        nc.vector.tensor_tensor(y_sb[:], po[:], rinv[:], ALU.mult)
                        g = NIT * b + it  # destination core / a2a block
                        nc.sync.dma_start(a2a_in[lh][g, :, :], y_sb[:])

                    for b in range(B):
                        kT = kT_sb[(lh, b)]
                        for it in range(NIT):
                            q_ap = qT_sb[(lh, b)][:, it * IT : (it + 1) * IT]
                            njc = (it + 1) * (IT // P)
                            ngr = njc // 2
                            po = psO.tile([P, IT], F32, tag="o", name=f"po{lh}{b}{it}")
                            pr = psR.tile([P, IT], F32, tag="r", name=f"pr{lh}{b}{it}")
                            # groups routed directly to the PE rowsum (no
                            # DVE): the last group; for it=0 both groups
                            direct = set() if ngr == 2 else {ngr - 1}
                            racc2 = small.tile(
                                [P, 2, IT],
                                BF16,
                                tag="racc",
                                name=f"ra{lh}{b}{it}",
                            )
                            first_direct = [True]

                            def emit_av(e_tile, jg):
                                for k2 in range(2):
                                    jc = jg * 2 + k2
                                    nc.tensor.matmul(
                                        po[:],
                                        lhsT=v_sb[b][:, jc, lh * DH : (lh + 1) * DH],
                                        rhs=e_tile[:, k2, :],
                                        start=(jc == 0),
                                        stop=(jc == njc - 1),
                                    )
                                if jg in direct:
                                    # rowsum straight on the PE, riding the
                                    # AV pipeline so exp/mask are long done
                                    for k2 in range(2):
                                        nc.tensor.matmul(
                                            pr[:],
                                            lhsT=ones_sb[:],
                                            rhs=e_tile[:, k2, :],
                                            start=first_direct[0],
                                            stop=False,
                                        )
                                        first_direct[0] = False

                            pipe = []
                            e_diag = None
                            for jg in range(ngr):
                                ps2 = psS.tile([P, 2, IT], F32, tag="s")
                                for k2 in range(2):
                                    jc = jg * 2 + k2
                                    nc.tensor.matmul(
                                        ps2[:, k2, :],
                                        lhsT=kT[:, jc * P : (jc + 1) * P],
                                        rhs=q_ap,
                                        start=True,
                                        stop=True,
                                    )
                                if jg == 0:
                                    # prev i-tile's rowsum matmuls slot in
                                    # behind this tile's first scores
                                    flush_pending()
                                if jg >= ngr - 2:
                                    # the two diagonal groups share one tile
                                    # so ONE batched [P,4,IT] mask op covers
                                    # them both
                                    if e_diag is None:
                                        e_diag = epool.tile(
                                            [P, 4, IT], BF16, tag="ed",
                                            bufs=2, name=f"ed{lh}{b}{it}"
                                        )
                                    half = jg - (ngr - 2)
                                    e_sb = e_diag[:, 2 * half : 2 * half + 2, :]
                                else:
                                    e_sb = epool.tile(
                                        [P, 2, IT], BF16, tag="e",
                                        bufs=3, name=f"e{lh}{b}{it}{jg}"
                                    )[:]
                                nc.scalar.activation(
                                    e_sb, ps2[:], AF.Exp, scale=1.0 / SCALE
                                )
                                if jg == ngr - 1:
                                    nc.vector.tensor_tensor(
                                        e_diag[:], e_diag[:], mask_sb[:], ALU.mult
                                    )
                                # rowsum accumulation on the DVE: groups 0+1
                                # combine in one op; later non-direct groups
                                # add pairwise.  The second-to-last (masked)
                                # group's add is deferred to after the mask.
                                if racc2 is not None:
                                    if jg == 1 and ngr == 2:
                                        # both groups are in e_diag; combine
                                        # after the mask op (DVE-ordered)
                                        nc.vector.tensor_tensor(
                                            racc2[:],
                                            e_diag[:, 0:2, :],
                                            e_diag[:, 2:4, :],
                                            ALU.add,
                                        )
                                    elif jg == 1:
                                        nc.vector.tensor_tensor(
                                            racc2[:], pipe[0][0], e_sb, ALU.add
                                        )
                                    elif jg == ngr - 1 and ngr > 2:
                                        nc.vector.tensor_tensor(
                                            racc2[:],
                                            racc2[:],
                                            e_diag[:, 0:2, :],
                                            ALU.add,
                                        )
                                    elif 1 < jg < ngr - 2:
                                        nc.vector.tensor_tensor(
                                            racc2[:], racc2[:], e_sb, ALU.add
                                        )
                                # AV lags two groups so the PE never waits
                                # on the exp/mask chain
                                pipe.append((e_sb, jg))
                                if len(pipe) > 2:
                                    emit_av(*pipe.pop(0))
                            for ent in pipe:
                                emit_av(*ent)
                            pending = (racc2, po, pr, b, it, bool(direct))
                    flush_pending()  # before this lh's collective
                    if pre_cc is not None:
                        pre_cc()
                nc.gpsimd.collective_compute(
                    "AllToAll",
                    ALU.bypass,
                    replica_groups=[list(range(NCORES))],
                    ins=[a2a_in[lh][:].opt()],
                    outs=[a2a_out[lh][:].opt()],
                )

            def ya_dma(lh):
                half = NCORES // 2
                ro = a2a_out[lh][:].rearrange("(s p) i -> p s i", p=P)
                nc.sync.dma_start(ya_sb[lh][:, :half, :], ro[:, :half, :])
                nc.sync.dma_start(ya_sb[lh][:, half:, :], ro[:, half:, :])

            proj_pass("a")
            attention(0)
            # ya_dma(0) is emitted AFTER att(1)'s a2a_in writes (pre_cc) so
            # the in-order Sync queue never blocks them on collective(0)
            attention(1, pre_cc=lambda: ya_dma(0))
            ya_dma(1)

            # ---------- output projection on own token slice ----------
            # ya_sb[lh] block s holds key chunk jc = 2s + lh, i.e. the s-th
            # chunk of wo_sb[lh] (parity-split).  ALL lh=0 matmuls run first
            # (partials staged to SBUF) so they cover the lh=1 AllToAll;
            # lh=1 matmuls then reuse the psum banks and the DVE combines
            # partial + psum + bias.  The two sub-chains are interleaved so
            # consecutive matmuls hit different psum banks.
            with (
                tc.tile_pool(name="opart", bufs=1) as opart,
                tc.tile_pool(name="ostage", bufs=2) as ostage,
                tc.tile_pool(name="psout", bufs=4, space="PSUM") as psout,
            ):
                EG = 2  # e-chunks per psum tile
                NEG = NDC // EG
                parts = [
                    opart.tile([P, EG, TSL], BF16, name=f"part{eg}")
                    for eg in range(NEG)
                ]

                def emit_mms(lh, eg, ps):
                    for s in range(NCORES):
                        for sub in range(EG):
                            ec = eg * EG + sub
                            nc.tensor.matmul(
                                ps[:, sub, :],
                                lhsT=wo_sb[lh][:, s, ec * P : ec * P + P],
                                rhs=ya_sb[lh][:, s, :],
                                start=(s == 0),
                                stop=(s == NCORES - 1),
                            )

                # pass 1: lh=0 into psum, drain raw partials to SBUF
                for eg in range(NEG):
                    ps = psout.tile([P, EG, TSL], F32, tag="out", name=f"p0_{eg}")
                    emit_mms(0, eg, ps)
                    nc.vector.tensor_copy(parts[eg][:], ps[:])
                # pass 2: lh=1 into psum, combine with partial + bias, store
                for eg in range(NEG):
                    ps = psout.tile([P, EG, TSL], F32, tag="out", name=f"p1_{eg}")
                    emit_mms(1, eg, ps)
                    ost = ostage.tile([P, EG, TSL], BF16, tag="ost", name=f"os{eg}")
                    nc.vector.tensor_tensor(ost[:], ps[:], parts[eg][:], ALU.add)
                    nc.vector.tensor_tensor(
                        ost[:],
                        ost[:],
                        bo_sb[:, eg * EG : (eg + 1) * EG, None].to_broadcast(
                            [P, EG, TSL]
                        ),
                        ALU.add,
                    )
                    nc.sync.dma_start(out[:, eg * EG : (eg + 1) * EG, :], ost[:])

    nc.finalize()  # bacc compile: regalloc etc. -- required before execution
    return nc


_PROGRAM = None


def _get_program():
    global _PROGRAM
    if _PROGRAM is None:
        _PROGRAM = build_program()
    return _PROGRAM


def _host_prep(x, mask, wq, bq, wk, bk, wv, bv, wo, bo):
    """Build the 8 per-core input maps (host-side marshalling only)."""
    import ml_dtypes

    f = np.float32
    bf = ml_dtypes.bfloat16
    x2 = np.asarray(x, dtype=f).reshape(T, D)
    # [ts][p][dc][t] blocked x^T so every DMA descriptor is contiguous
    xb = x2.T.reshape(NDC, P, NTS, IT).transpose(2, 1, 0, 3).astype(bf)

    # fold the v-bias through the output projection: softmax rows sum to 1
    # so attn@(v+bv) @ wo^T + bo == attn@v @ wo^T + (wo @ bv + bo)
    bo_eff = np.asarray(bo, dtype=f) + np.asarray(wo, dtype=f) @ np.asarray(bv, dtype=f)
    woT = np.asarray(wo, dtype=f).T.reshape(NJC, P, D)  # [jc][p][e]
    woE = woT[0::2].transpose(1, 0, 2).astype(bf)  # [p][s][e], jc = 2s
    woO = woT[1::2].transpose(1, 0, 2).astype(bf)  # [p][s][e], jc = 2s+1
    bo_b = np.ascontiguousarray(bo_eff.reshape(NDC, P).T)

    # diagonal-band mask patterns from the provided mask (True = masked out)
    mask_np = np.asarray(mask)
    maskp = np.empty((4, P, IT), dtype=f)
    for m in range(4):
        maskp[m] = (~mask_np[0:IT, m * P : (m + 1) * P]).T.astype(f)
    maskb = maskp.transpose(1, 0, 2).astype(bf)  # [p][m][i]

    wq_, wk_, wv_ = (np.asarray(w, dtype=f) for w in (wq, wk, wv))
    bq_, bk_ = (np.asarray(v_, dtype=f) for v_ in (bq, bk))

    in_maps = []
    for c in range(NCORES):
        j0, j1 = c * JW, (c + 1) * JW
        in_maps.append(
            {
                "xb": xb,
                "wqb": wq_[j0:j1, :].T.reshape(NDC, P, JW).transpose(1, 0, 2).astype(bf),
                "wkb": wk_[j0:j1, :].T.reshape(NDC, P, JW).transpose(1, 0, 2).astype(bf),
                "wvb": wv_[j0:j1, :].T.reshape(NDC, P, JW).transpose(1, 0, 2).astype(bf),
                "woE": woE,
                "woO": woO,
                "bqb": np.ascontiguousarray(bq_[j0:j1].reshape(HPC, P).T),
                "bkb": np.ascontiguousarray(bk_[j0:j1].reshape(HPC, P).T),
                "bob": bo_b,
                "maskb": maskb,
                "onesb": np.ones((P, P), dtype=bf),
            }
        )
    return in_maps


LAST_RESULTS = None  # BassKernelResults of the most recent run (for test.py)


def _assemble(per_core_outs):
    """[P, NDC, TSL] blocked slices -> full [B, S, D] output."""
    outT = np.concatenate(
        [
            np.asarray(o, dtype=np.float32)
            .reshape(P, NDC, TSL)
            .transpose(1, 0, 2)
            .reshape(D, TSL)
            for o in per_core_outs
        ],
        axis=1,
    )
    return np.ascontiguousarray(outT.T).reshape(B, S, D).astype(np.float32)


def kernel(x, mask, wq, bq, wk, bk, wv, bv, wo, bo):
    global LAST_RESULTS
    from concourse.bass_utils import run_bass_kernel_spmd

    nc = _get_program()
    in_maps = _host_prep(x, mask, wq, bq, wk, bk, wv, bv, wo, bo)
    trace = os.environ.get("KERNEL_TRACE", "") == "1"
    kwargs = {}
    if os.environ.get("KERNEL_TRACE_ALL", "") == "1":
        kwargs["trace_cores"] = list(range(NCORES))
        kwargs["stitch_traces"] = True
    res = run_bass_kernel_spmd(
        nc, in_maps, core_ids=list(range(NCORES)), trace=trace, **kwargs
    )
    LAST_RESULTS = res
    return _assemble([res.results[c]["out"] for c in range(NCORES)])


# revision 28
# speedup vs baseline: 1.0636x; 1.0636x over previous
"""Trainium2 Bass kernel for nn_MultiHeadAttention (B=2, S=2048, D=2048, H=16).

Sharding: tensor-parallel over heads -- each of the 8 cores owns 2 heads
(both batches) for the q/k/v projections and attention, then two 8-way
AllToAlls (one per local head) convert the head-sharded attention output
Y^T into a token-sharded layout, and each core computes a disjoint
512-token slice of the output projection (no all-reduce needed).

Key structure (evolved against neuron-profile traces):
- All matmul operands are bf16; psum accumulation stays f32.  The PE
  sustains ~0.515 ns/col in collective-bearing programs (1.94 GHz;
  CC-free microbenches run at 2.4 GHz -- collectives throttle the PE
  ~18% program-wide, and the SWDGE remote_dma path that would avoid
  them crashes this runtime).  ~710k matmul cols/core => ~366 us PE
  floor; everything else hides behind it.
- q^T / k^T / v live in SBUF between phases; host pre-blocks all DRAM
  inputs into the exact [partition][...] layouts (contiguous DMAs).
- Projection streams x once (x tiles DMA'd BEFORE the 6 MB of weights
  so they are not starved on HBM bandwidth); 8 interleaved psum chains;
  V chains are double-buffered and run VLAG dc ahead so slice
  boundaries never wait on the q/k psum drains.
- Attention epilogue balance (per head-batch: PE 25us, DVE ~24, ACT 17):
  exp groups accumulate pairwise into a [P,2,IT] bf16 racc2 on the DVE
  (groups 0+1 combine in one op; the last group contracts directly on
  the PE, riding the AV pipeline); the two diagonal groups share one
  [P,4,IT] tile so a single batched DVE op applies the causal mask; AV
  emission lags two groups so the PE never waits on the exp/mask chain.
  V-bias and output bias fold into bo_eff = bo + wo@bv on the host
  (softmax rows sum to 1).
- A tiny warmup AllToAll during the projections absorbs the ~11 us CC
  arming cost so AllToAll(0) starts ~1 us after its trigger.
- Phase order: proj -> att(lh0) -> AllToAll(0) -> att(lh1) -> ya0
  readback -> AllToAll(1) -> ya1 readback -> out-proj.  att(lh1) covers
  AllToAll(0); out-proj pass 1 (all lh0 matmuls, partials staged in
  SBUF) covers AllToAll(1).  ya_dma(0) is emitted AFTER att(lh1)'s
  a2a_in writes so the in-order Sync queue never blocks them (the
  baseline lost 23 us + an out-proj DVFS ramp to that).
- Out-proj interleaves its two psum sub-chains (LDWEIGHTS stays hidden)
  and stages the output in bf16 (host upcasts) to halve the final DMA.
"""

import os
import sys

import numpy as np

_REPO = "/opt/trn_rl_repo"
if _REPO not in sys.path:
    sys.path.insert(0, _REPO)

from concourse import bacc, mybir, tile  # noqa: E402
import concourse.bass as bass  # noqa: E402

B, S, D, H = 2, 2048, 2048, 16
DH = D // H  # 128
NCORES = 8
HPC = H // NCORES  # heads per core = 2
JW = HPC * DH  # per-core head-feature width = 256
T = B * S  # 4096 flattened tokens
TSL = T // NCORES  # per-core output token slice = 512
SCALE = float(np.sqrt(DH))

F32 = mybir.dt.float32
BF16 = mybir.dt.bfloat16
AF = mybir.ActivationFunctionType
ALU = bass.mybir.AluOpType

P = 128
IT = 512  # query i-tile width
NIT = S // IT  # 4 i-tiles per (batch, head)
NJC = S // P  # 16 key chunks per batch
NDC = D // P  # 16 contraction chunks
NTS = T // IT  # 8 token slices (batch 0 first, then batch 1)
XSUB = 8  # x dc-chunks per sub-tile (few big DMAs win)
NXS = NDC // XSUB  # 2 x sub-tiles per slice
VLAG = 6  # V chains run this many dc ahead (slice-boundary warmup)


def build_program():
    nc = bacc.Bacc(
        "TRN2",
        target_bir_lowering=False,
        debug=False,
        num_devices=NCORES,
    )

    # ---- kernel I/O (host pre-blocked; per-core values via in_maps) ----
    xb = nc.dram_tensor("xb", [NTS, P, NDC, IT], BF16, kind="ExternalInput").ap()
    wqb = nc.dram_tensor("wqb", [P, NDC, JW], BF16, kind="ExternalInput").ap()
    wkb = nc.dram_tensor("wkb", [P, NDC, JW], BF16, kind="ExternalInput").ap()
    wvb = nc.dram_tensor("wvb", [P, NDC, JW], BF16, kind="ExternalInput").ap()
    # wo split by key-chunk parity: even chunks feed lh=0, odd feed lh=1
    woE = nc.dram_tensor("woE", [P, NJC // 2, D], BF16, kind="ExternalInput").ap()
    woO = nc.dram_tensor("woO", [P, NJC // 2, D], BF16, kind="ExternalInput").ap()
    bqb = nc.dram_tensor("bqb", [P, HPC], F32, kind="ExternalInput").ap()
    bkb = nc.dram_tensor("bkb", [P, HPC], F32, kind="ExternalInput").ap()
    bob = nc.dram_tensor("bob", [P, NDC], F32, kind="ExternalInput").ap()
    # 4 diagonal-band mask patterns (1.0 = attend), [p][m][i]
    maskb = nc.dram_tensor("maskb", [P, 4, IT], BF16, kind="ExternalInput").ap()
    onesb = nc.dram_tensor("onesb", [P, P], BF16, kind="ExternalInput").ap()
    out = nc.dram_tensor("out", [P, NDC, TSL], BF16, kind="ExternalOutput").ap()

    with tile.TileContext(nc) as tc:
        with (
            tc.tile_pool(name="dram", bufs=1, space="DRAM") as dram,
            tc.tile_pool(name="const", bufs=1) as cpool,
            tc.tile_pool(name="persist", bufs=1) as ppool,
            tc.tile_pool(name="small", bufs=2) as small,
            tc.tile_pool(name="epool", bufs=2) as epool,
        ):
            # ---- persistent SBUF tiles ----
            qT_sb = {}
            kT_sb = {}
            for lh in range(HPC):
                for b in range(B):
                    qT_sb[(lh, b)] = ppool.tile([P, S], BF16, name=f"qT_{lh}_{b}")
                    kT_sb[(lh, b)] = ppool.tile([P, S], BF16, name=f"kT_{lh}_{b}")
            v_sb = {
                b: ppool.tile([P, NJC, JW], BF16, name=f"v_{b}") for b in range(B)
            }
            ya_sb = {
                lh: ppool.tile([P, NCORES, TSL], BF16, name=f"ya{lh}")
                for lh in range(HPC)
            }
            wo_sb = {
                0: ppool.tile([P, NJC // 2, D], BF16, name="woE"),
                1: ppool.tile([P, NJC // 2, D], BF16, name="woO"),
            }

            # per-local-head AllToAll buffers (blocks = dest core's i-slice)
            a2a_in = {
                lh: dram.tile([NCORES, DH, TSL], BF16, name=f"a2a_in_{lh}")
                for lh in range(HPC)
            }
            a2a_out = {
                lh: dram.tile([NCORES * DH, TSL], BF16, name=f"a2a_out_{lh}")
                for lh in range(HPC)
            }

            # tiny warmup AllToAll: absorbs the ~11 us CC arming cost
            # while the projections run, so the real AllToAll(0) starts fast
            a2aw_in = dram.tile([NCORES, P, 4], BF16, name="a2aw_in")
            a2aw_out = dram.tile([NCORES * P, 4], BF16, name="a2aw_out")

            # ---- constants / weights ----
            wq_w = cpool.tile([P, NDC, JW], BF16)
            wk_w = cpool.tile([P, NDC, JW], BF16)
            wv_w = cpool.tile([P, NDC, JW], BF16)
            bq_sb = cpool.tile([P, HPC], F32)
            bk_sb = cpool.tile([P, HPC], F32)
            bo_sb = cpool.tile([P, NDC], F32)
            mask_sb = cpool.tile([P, 4, IT], BF16)
            ones_sb = cpool.tile([P, P], BF16)

            # warmup AllToAll emission: DVE memset + gpsimd-queue DMAs so
            # the Sync queue (x/weight streaming) is untouched
            wsrc = cpool.tile([P, 4], BF16, name="wsrc")
            nc.vector.memset(wsrc[:], 1.0)
            for gq in range(NCORES):
                nc.gpsimd.dma_start(a2aw_in[gq, :, :], wsrc[:])
            nc.gpsimd.collective_compute(
                "AllToAll",
                ALU.bypass,
                replica_groups=[list(range(NCORES))],
                ins=[a2aw_in[:].opt()],
                outs=[a2aw_out[:].opt()],
            )

            # ---------- projections (SBUF-resident outputs) ----------
            def proj_pass(tag):
                """q/k projections for both heads + V, streamed over the
                8 token slices.  V chains (bufs=2) run VLAG dc ahead so the
                slice boundary never waits on the q/k psum drains."""
                with (
                    tc.tile_pool(name=f"xpool{tag}", bufs=1) as xpool,
                    tc.tile_pool(name=f"psum_{tag}", bufs=1, space="PSUM") as psp,
                ):
                    for ts in range(NTS):
                        b, lt0 = ts // NIT, (ts % NIT) * IT
                        xs = []
                        for g in range(NXS):
                            xg = xpool.tile(
                                [P, XSUB, IT],
                                BF16,
                                tag="x",
                                bufs=2,
                                name=f"x{tag}_{ts}_{g}",
                            )
                            if ts == 0 and g == 0:
                                nc.sync.dma_start(
                                    xg[:, 0:2, :], xb[ts, :, 0:2, :]
                                )
                                nc.sync.dma_start(
                                    xg[:, 2:XSUB, :], xb[ts, :, 2:XSUB, :]
                                )
                            else:
                                nc.sync.dma_start(
                                    xg[:], xb[ts, :, g * XSUB : (g + 1) * XSUB, :]
                                )
                            xs.append(xg)
                        if ts == 0:
                            # weights AFTER ts0's x tiles so x isn't queued
                            # behind 6 MB on the shared HBM bandwidth
                            nc.sync.dma_start(wv_w[:, 0:4, :], wvb[:, 0:4, :])
                            nc.sync.dma_start(wk_w[:, 0:2, :], wkb[:, 0:2, :])
                            nc.sync.dma_start(wq_w[:, 0:2, :], wqb[:, 0:2, :])
                            nc.sync.dma_start(wv_w[:, 4:8, :], wvb[:, 4:8, :])
                            nc.sync.dma_start(wk_w[:, 2:5, :], wkb[:, 2:5, :])
                            nc.sync.dma_start(wq_w[:, 2:5, :], wqb[:, 2:5, :])
                            nc.sync.dma_start(wk_w[:, 5:8, :], wkb[:, 5:8, :])
                            nc.sync.dma_start(wq_w[:, 5:8, :], wqb[:, 5:8, :])
                            nc.sync.dma_start(wk_w[:, 8:12, :], wkb[:, 8:12, :])
                            nc.sync.dma_start(wq_w[:, 8:12, :], wqb[:, 8:12, :])
                            nc.sync.dma_start(wk_w[:, 12:16, :], wkb[:, 12:16, :])
                            nc.sync.dma_start(wq_w[:, 12:16, :], wqb[:, 12:16, :])
                            nc.sync.dma_start(wv_w[:, 8:16, :], wvb[:, 8:16, :])
                            nc.sync.dma_start(bk_sb[:], bkb)
                            nc.sync.dma_start(bq_sb[:], bqb)
                            nc.sync.dma_start(bo_sb[:], bob)
                            nc.sync.dma_start(mask_sb[:], maskb)
                            nc.sync.dma_start(ones_sb[:], onesb)
                        if ts == 2:
                            nc.sync.dma_start(wo_sb[0][:], woE)
                        if ts == 4:
                            nc.sync.dma_start(wo_sb[1][:], woO)

                        def xchunk(dc):
                            return xs[dc // XSUB][:, dc % XSUB, :]

                        pqk = {
                            nm: psp.tile(
                                [P, IT], F32, tag=nm, name=f"p{nm}_{ts}"
                            )
                            for nm in ("k0", "k1", "q0", "q1")
                        }
                        pv = {
                            half: psp.tile(
                                [P, 2, JW],
                                F32,
                                tag=f"v{half}",
                                bufs=2,
                                name=f"pv{ts}{half}",
                            )
                            for half in range(2)
                        }

                        def emit_v(dc):
                            sp = dc == NDC - 1
                            for tc2 in range(IT // P):
                                nc.tensor.matmul(
                                    pv[tc2 // 2][:, tc2 % 2, :],
                                    lhsT=xchunk(dc)[:, tc2 * P : (tc2 + 1) * P],
                                    rhs=wv_w[:, dc, :],
                                    start=(dc == 0 and tc2 % 2 == 0),
                                    stop=sp,
                                    skip_group_check=True,
                                )

                        # V warmup: first VLAG dc of the V chains keep the
                        # PE busy while this slice's q/k psum banks drain
                        for dc in range(VLAG):
                            emit_v(dc)
                        for dc in range(NDC):
                            st, sp = dc == 0, dc == NDC - 1
                            for h in range(HPC):
                                nc.tensor.matmul(
                                    pqk[f"k{h}"][:],
                                    lhsT=wk_w[:, dc, h * DH : (h + 1) * DH],
                                    rhs=xchunk(dc),
                                    start=st,
                                    stop=sp,
                                )
                                nc.tensor.matmul(
                                    pqk[f"q{h}"][:],
                                    lhsT=wq_w[:, dc, h * DH : (h + 1) * DH],
                                    rhs=xchunk(dc),
                                    start=st,
                                    stop=sp,
                                )
                            if dc < NDC - VLAG:
                                emit_v(dc + VLAG)
                        # epilogues on DVE: bias add, write bf16 persistents
                        for h in range(HPC):
                            nc.vector.tensor_tensor(
                                kT_sb[(h, b)][:, lt0 : lt0 + IT],
                                pqk[f"k{h}"][:],
                                bk_sb[:, h : h + 1].to_broadcast([P, IT]),
                                ALU.add,
                            )
                            nc.vector.tensor_tensor(
                                qT_sb[(h, b)][:, lt0 : lt0 + IT],
                                pqk[f"q{h}"][:],
                                bq_sb[:, h : h + 1].to_broadcast([P, IT]),
                                ALU.add,
                            )
                        for half in range(2):
                            # v bias folded into bo on the host
                            nc.vector.tensor_copy(
                                v_sb[b][
                                    :,
                                    lt0 // P + 2 * half : lt0 // P + 2 * half + 2,
                                    :,
                                ],
                                pv[half][:],
                            )

            # ---------- attention for one local head + its AllToAll ----------
            def attention(lh, pre_cc=None):
                with (
                    tc.tile_pool(name=f"psS{lh}", bufs=2, space="PSUM") as psS,
                    tc.tile_pool(name=f"psO{lh}", bufs=2, space="PSUM") as psO,
                    tc.tile_pool(name=f"psR{lh}", bufs=2, space="PSUM") as psR,
                ):
                    # softmax denominators: exp groups accumulate pairwise
                    # into racc2 [P,2,IT] on the DVE (one op per group),
                    # contracted by TWO ones-matmuls per i-tile.  That
                    # contraction + epilogue are emitted after the next
                    # tile's first scores group so the PE never waits on
                    # the DVE tail.
                    pending = None

                    def flush_pending():
                        nonlocal pending
                        if pending is None:
                            return
                        racc2, po, pr, b, it, had_direct = pending
                        pending = None
                        if racc2 is not None:
                            nc.tensor.matmul(
                                pr[:],
                                lhsT=ones_sb[:],
                                rhs=racc2[:, 0, :],
                                start=not had_direct,
                                stop=False,
                            )
                            nc.tensor.matmul(
                                pr[:],
                                lhsT=ones_sb[:],
                                rhs=racc2[:, 1, :],
                                start=False,
                                stop=True,
                            )
                        rinv = small.tile(
                            [P, IT], F32, tag="rinv", name=f"ri{lh}{b}{it}"
                        )
                        nc.vector.reciprocal_approx_fast(rinv[:], pr[:])
                        # v-bias and output bias are folded into bo on the
                        # host (softmax rows sum to 1), so y is just po/r
                        y_sb = small.tile(
                            [P, IT], BF16, tag="y", bufs=8, name=f"y{lh}{b}{it}"
                        )
                        nc.vector.tensor_tensor(y_sb[:], po[:], rinv[:], ALU.mult)
                        g = NIT * b + it  # destination core / a2a block
                        nc.sync.dma_start(a2a_in[lh][g, :, :], y_sb[:])

                    for b in range(B):
                        kT = kT_sb[(lh, b)]
                        for it in range(NIT):
                            q_ap = qT_sb[(lh, b)][:, it * IT : (it + 1) * IT]
                            njc = (it + 1) * (IT // P)
                            ngr = njc // 2
                            po = psO.tile([P, IT], F32, tag="o", name=f"po{lh}{b}{it}")
                            pr = psR.tile([P, IT], F32, tag="r", name=f"pr{lh}{b}{it}")
                            # groups routed directly to the PE rowsum (no
                            # DVE): the last group; for it=0 both groups
                            d_all = b == B - 1 and it == NIT - 1
                            if d_all:
                                # the tile that gates this head's AllToAll:
                                # whole denominator from direct PE matmuls,
                                # no DVE dependency in the trigger tail
                                direct = set(range(ngr))
                            elif ngr == 2:
                                direct = set()
                            else:
                                direct = {ngr - 1}
                            racc2 = (
                                None
                                if d_all
                                else small.tile(
                                    [P, 2, IT],
                                    BF16,
                                    tag="racc",
                                    name=f"ra{lh}{b}{it}",
                                )
                            )
                            first_direct = [True]

                            def emit_av(e_tile, jg):
                                for k2 in range(2):
                                    jc = jg * 2 + k2
                                    nc.tensor.matmul(
                                        po[:],
                                        lhsT=v_sb[b][:, jc, lh * DH : (lh + 1) * DH],
                                        rhs=e_tile[:, k2, :],
                                        start=(jc == 0),
                                        stop=(jc == njc - 1),
                                    )
                                if jg in direct:
                                    # rowsum straight on the PE, riding the
                                    # AV pipeline so exp/mask are long done
                                    for k2 in range(2):
                                        nc.tensor.matmul(
                                            pr[:],
                                            lhsT=ones_sb[:],
                                            rhs=e_tile[:, k2, :],
                                            start=first_direct[0],
                                            stop=(
                                                racc2 is None
                                                and jg == ngr - 1
                                                and k2 == 1
                                            ),
                                        )
                                        first_direct[0] = False

                            pipe = []
                            e_diag = None
                            for jg in range(ngr):
                                ps2 = psS.tile([P, 2, IT], F32, tag="s")
                                for k2 in range(2):
                                    jc = jg * 2 + k2
                                    nc.tensor.matmul(
                                        ps2[:, k2, :],
                                        lhsT=kT[:, jc * P : (jc + 1) * P],
                                        rhs=q_ap,
                                        start=True,
                                        stop=True,
                                    )
                                if jg == 0:
                                    # prev i-tile's rowsum matmuls slot in
                                    # behind this tile's first scores
                                    flush_pending()
                                if jg >= ngr - 2:
                                    # the two diagonal groups share one tile
                                    # so ONE batched [P,4,IT] mask op covers
                                    # them both
                                    if e_diag is None:
                                        e_diag = epool.tile(
                                            [P, 4, IT], BF16, tag="ed",
                                            bufs=2, name=f"ed{lh}{b}{it}"
                                        )
                                    half = jg - (ngr - 2)
                                    e_sb = e_diag[:, 2 * half : 2 * half + 2, :]
                                else:
                                    e_sb = epool.tile(
                                        [P, 2, IT], BF16, tag="e",
                                        bufs=3, name=f"e{lh}{b}{it}{jg}"
                                    )[:]
                                nc.scalar.activation(
                                    e_sb, ps2[:], AF.Exp, scale=1.0 / SCALE
                                )
                                if jg == ngr - 1:
                                    nc.vector.tensor_tensor(
                                        e_diag[:], e_diag[:], mask_sb[:], ALU.mult
                                    )
                                # rowsum accumulation on the DVE: groups 0+1
                                # combine in one op; later non-direct groups
                                # add pairwise.  The second-to-last (masked)
                                # group's add is deferred to after the mask.
                                if racc2 is not None:
                                    if jg == 1 and ngr == 2:
                                        # both groups are in e_diag; combine
                                        # after the mask op (DVE-ordered)
                                        nc.vector.tensor_tensor(
                                            racc2[:],
                                            e_diag[:, 0:2, :],
                                            e_diag[:, 2:4, :],
                                            ALU.add,
                                        )
                                    elif jg == 1:
                                        nc.vector.tensor_tensor(
                                            racc2[:], pipe[0][0], e_sb, ALU.add
                                        )
                                    elif jg == ngr - 1 and ngr > 2:
                                        nc.vector.tensor_tensor(
                                            racc2[:],
                                            racc2[:],
                                            e_diag[:, 0:2, :],
                                            ALU.add,
                                        )
                                    elif 1 < jg < ngr - 2:
                                        nc.vector.tensor_tensor(
                                            racc2[:], racc2[:], e_sb, ALU.add
                                        )
                                # AV lags two groups so the PE never waits
                                # on the exp/mask chain
                                pipe.append((e_sb, jg))
                                if len(pipe) > 2:
                                    emit_av(*pipe.pop(0))
                            for ent in pipe:
                                emit_av(*ent)
                            pending = (racc2, po, pr, b, it, bool(direct))
                    flush_pending()  # before this lh's collective
                    if pre_cc is not None:
                        pre_cc()
                nc.gpsimd.collective_compute(
                    "AllToAll",
                    ALU.bypass,
                    replica_groups=[list(range(NCORES))],
                    ins=[a2a_in[lh][:].opt()],
                    outs=[a2a_out[lh][:].opt()],
                )

            def ya_dma(lh):
                half = NCORES // 2
                ro = a2a_out[lh][:].rearrange("(s p) i -> p s i", p=P)
                nc.sync.dma_start(ya_sb[lh][:, :half, :], ro[:, :half, :])
                nc.sync.dma_start(ya_sb[lh][:, half:, :], ro[:, half:, :])

            proj_pass("a")
            attention(0)
            # ya_dma(0) is emitted AFTER att(1)'s a2a_in writes (pre_cc) so
            # the in-order Sync queue never blocks them on collective(0)
            attention(1, pre_cc=lambda: ya_dma(0))
            ya_dma(1)

            # ---------- output projection on own token slice ----------
            # ya_sb[lh] block s holds key chunk jc = 2s + lh, i.e. the s-th
            # chunk of wo_sb[lh] (parity-split).  ALL lh=0 matmuls run first
            # (partials staged to SBUF) so they cover the lh=1 AllToAll;
            # lh=1 matmuls then reuse the psum banks and the DVE combines
            # partial + psum + bias.  The two sub-chains are interleaved so
            # consecutive matmuls hit different psum banks.
            with (
                tc.tile_pool(name="opart", bufs=1) as opart,
                tc.tile_pool(name="ostage", bufs=2) as ostage,
                tc.tile_pool(name="psout", bufs=4, space="PSUM") as psout,
            ):
                EG = 2  # e-chunks per psum tile
                NEG = NDC // EG
                parts = [
                    opart.tile([P, EG, TSL], BF16, name=f"part{eg}")
                    for eg in range(NEG)
                ]

                def emit_mms(lh, eg, ps):
                    for s in range(NCORES):
                        for sub in range(EG):
                            ec = eg * EG + sub
                            nc.tensor.matmul(
                                ps[:, sub, :],
                                lhsT=wo_sb[lh][:, s, ec * P : ec * P + P],
                                rhs=ya_sb[lh][:, s, :],
                                start=(s == 0),
                                stop=(s == NCORES - 1),
                            )

                # pass 1: lh=0 into psum, drain raw partials to SBUF
                for eg in range(NEG):
                    ps = psout.tile([P, EG, TSL], F32, tag="out", name=f"p0_{eg}")
                    emit_mms(0, eg, ps)
                    nc.vector.tensor_copy(parts[eg][:], ps[:])
                # pass 2: lh=1 into psum, combine with partial + bias, store
                for eg in range(NEG):
                    ps = psout.tile([P, EG, TSL], F32, tag="out", name=f"p1_{eg}")
                    emit_mms(1, eg, ps)
                    ost = ostage.tile([P, EG, TSL], BF16, tag="ost", name=f"os{eg}")
                    nc.vector.tensor_tensor(ost[:], ps[:], parts[eg][:], ALU.add)
                    nc.vector.tensor_tensor(
                        ost[:],
                        ost[:],
                        bo_sb[:, eg * EG : (eg + 1) * EG, None].to_broadcast(
                            [P, EG, TSL]
                        ),
                        ALU.add,
                    )
                    nc.sync.dma_start(out[:, eg * EG : (eg + 1) * EG, :], ost[:])

    nc.finalize()  # bacc compile: regalloc etc. -- required before execution
    return nc


_PROGRAM = None


def _get_program():
    global _PROGRAM
    if _PROGRAM is None:
        _PROGRAM = build_program()
    return _PROGRAM


def _host_prep(x, mask, wq, bq, wk, bk, wv, bv, wo, bo):
    """Build the 8 per-core input maps (host-side marshalling only)."""
    import ml_dtypes

    f = np.float32
    bf = ml_dtypes.bfloat16
    x2 = np.asarray(x, dtype=f).reshape(T, D)
    # [ts][p][dc][t] blocked x^T so every DMA descriptor is contiguous
    xb = x2.T.reshape(NDC, P, NTS, IT).transpose(2, 1, 0, 3).astype(bf)

    # fold the v-bias through the output projection: softmax rows sum to 1
    # so attn@(v+bv) @ wo^T + bo == attn@v @ wo^T + (wo @ bv + bo)
    bo_eff = np.asarray(bo, dtype=f) + np.asarray(wo, dtype=f) @ np.asarray(bv, dtype=f)
    woT = np.asarray(wo, dtype=f).T.reshape(NJC, P, D)  # [jc][p][e]
    woE = woT[0::2].transpose(1, 0, 2).astype(bf)  # [p][s][e], jc = 2s
    woO = woT[1::2].transpose(1, 0, 2).astype(bf)  # [p][s][e], jc = 2s+1
    bo_b = np.ascontiguousarray(bo_eff.reshape(NDC, P).T)

    # diagonal-band mask patterns from the provided mask (True = masked out)
    mask_np = np.asarray(mask)
    maskp = np.empty((4, P, IT), dtype=f)
    for m in range(4):
        maskp[m] = (~mask_np[0:IT, m * P : (m + 1) * P]).T.astype(f)
    maskb = maskp.transpose(1, 0, 2).astype(bf)  # [p][m][i]

    wq_, wk_, wv_ = (np.asarray(w, dtype=f) for w in (wq, wk, wv))
    bq_, bk_ = (np.asarray(v_, dtype=f) for v_ in (bq, bk))

    in_maps = []
    for c in range(NCORES):
        j0, j1 = c * JW, (c + 1) * JW
        in_maps.append(
            {
                "xb": xb,
                "wqb": wq_[j0:j1, :].T.reshape(NDC, P, JW).transpose(1, 0, 2).astype(bf),
                "wkb": wk_[j0:j1, :].T.reshape(NDC, P, JW).transpose(1, 0, 2).astype(bf),
                "wvb": wv_[j0:j1, :].T.reshape(NDC, P, JW).transpose(1, 0, 2).astype(bf),
                "woE": woE,
                "woO": woO,
                "bqb": np.ascontiguousarray(bq_[j0:j1].reshape(HPC, P).T),
                "bkb": np.ascontiguousarray(bk_[j0:j1].reshape(HPC, P).T),
                "bob": bo_b,
                "maskb": maskb,
                "onesb": np.ones((P, P), dtype=bf),
            }
        )
    return in_maps


LAST_RESULTS = None  # BassKernelResults of the most recent run (for test.py)


def _assemble(per_core_outs):
    """[P, NDC, TSL] blocked slices -> full [B, S, D] output."""
    outT = np.concatenate(
        [
            np.asarray(o, dtype=np.float32)
            .reshape(P, NDC, TSL)
            .transpose(1, 0, 2)
            .reshape(D, TSL)
            for o in per_core_outs
        ],
        axis=1,
    )
    return np.ascontiguousarray(outT.T).reshape(B, S, D).astype(np.float32)


def kernel(x, mask, wq, bq, wk, bk, wv, bv, wo, bo):
    global LAST_RESULTS
    from concourse.bass_utils import run_bass_kernel_spmd

    nc = _get_program()
    in_maps = _host_prep(x, mask, wq, bq, wk, bk, wv, bv, wo, bo)
    trace = os.environ.get("KERNEL_TRACE", "") == "1"
    kwargs = {}
    if os.environ.get("KERNEL_TRACE_ALL", "") == "1":
        kwargs["trace_cores"] = list(range(NCORES))
        kwargs["stitch_traces"] = True
    res = run_bass_kernel_spmd(
        nc, in_maps, core_ids=list(range(NCORES)), trace=trace, **kwargs
    )
    LAST_RESULTS = res
    return _assemble([res.results[c]["out"] for c in range(NCORES)])


# revision 29
# speedup vs baseline: 1.0824x; 1.0177x over previous
"""Trainium2 Bass kernel for nn_MultiHeadAttention (B=2, S=2048, D=2048, H=16).

Sharding: tensor-parallel over heads -- each of the 8 cores owns 2 heads
(both batches) for the q/k/v projections and attention, then two 8-way
AllToAlls (one per local head) convert the head-sharded attention output
Y^T into a token-sharded layout, and each core computes a disjoint
512-token slice of the output projection (no all-reduce needed).

Key structure (evolved against neuron-profile traces):
- All matmul operands are bf16; psum accumulation stays f32.  The PE
  sustains ~0.515 ns/col in collective-bearing programs (1.94 GHz;
  CC-free microbenches run at 2.4 GHz -- collectives throttle the PE
  ~18% program-wide, and the SWDGE remote_dma path that would avoid
  them crashes this runtime).  ~710k matmul cols/core => ~366 us PE
  floor; everything else hides behind it.
- q^T / k^T / v live in SBUF between phases; host pre-blocks all DRAM
  inputs into the exact [partition][...] layouts (contiguous DMAs).
- Projection streams x once (x tiles DMA'd BEFORE the 6 MB of weights
  so they are not starved on HBM bandwidth); 8 interleaved psum chains;
  V chains are double-buffered and run VLAG dc ahead so slice
  boundaries never wait on the q/k psum drains.
- Attention epilogue balance (per head-batch: PE 25us, DVE ~24, ACT 17):
  exp groups accumulate pairwise into a [P,2,IT] bf16 racc2 on the DVE
  (groups 0+1 combine in one op; the last group contracts directly on
  the PE, riding the AV pipeline); the two diagonal groups share one
  [P,4,IT] tile so a single batched DVE op applies the causal mask; AV
  emission lags two groups so the PE never waits on the exp/mask chain.
  V-bias and output bias fold into bo_eff = bo + wo@bv on the host
  (softmax rows sum to 1).
- A tiny warmup AllToAll during the projections absorbs the ~11 us CC
  arming cost so AllToAll(0) starts ~1 us after its trigger.
- Phase order: proj -> att(lh0) -> AllToAll(0) -> att(lh1) -> ya0
  readback -> AllToAll(1) -> ya1 readback -> out-proj.  att(lh1) covers
  AllToAll(0); out-proj pass 1 (all lh0 matmuls, partials staged in
  SBUF) covers AllToAll(1).  ya_dma(0) is emitted AFTER att(lh1)'s
  a2a_in writes so the in-order Sync queue never blocks them (the
  baseline lost 23 us + an out-proj DVFS ramp to that).
- Out-proj interleaves its two psum sub-chains (LDWEIGHTS stays hidden)
  and stages the output in bf16 (host upcasts) to halve the final DMA.
"""

import os
import sys

import numpy as np

_REPO = "/opt/trn_rl_repo"
if _REPO not in sys.path:
    sys.path.insert(0, _REPO)

from concourse import bacc, mybir, tile  # noqa: E402
import concourse.bass as bass  # noqa: E402

B, S, D, H = 2, 2048, 2048, 16
DH = D // H  # 128
NCORES = 8
HPC = H // NCORES  # heads per core = 2
JW = HPC * DH  # per-core head-feature width = 256
T = B * S  # 4096 flattened tokens
TSL = T // NCORES  # per-core output token slice = 512
SCALE = float(np.sqrt(DH))

F32 = mybir.dt.float32
BF16 = mybir.dt.bfloat16
AF = mybir.ActivationFunctionType
ALU = bass.mybir.AluOpType

P = 128
IT = 512  # query i-tile width
NIT = S // IT  # 4 i-tiles per (batch, head)
NJC = S // P  # 16 key chunks per batch
NDC = D // P  # 16 contraction chunks
NTS = T // IT  # 8 token slices (batch 0 first, then batch 1)
XSUB = 8  # x dc-chunks per sub-tile (few big DMAs win)
NXS = NDC // XSUB  # 2 x sub-tiles per slice
VLAG = 6  # V chains run this many dc ahead (slice-boundary warmup)


def build_program():
    nc = bacc.Bacc(
        "TRN2",
        target_bir_lowering=False,
        debug=False,
        num_devices=NCORES,
    )

    # ---- kernel I/O (host pre-blocked; per-core values via in_maps) ----
    xb = nc.dram_tensor("xb", [NTS, P, NDC, IT], BF16, kind="ExternalInput").ap()
    wqb = nc.dram_tensor("wqb", [P, NDC, JW], BF16, kind="ExternalInput").ap()
    wkb = nc.dram_tensor("wkb", [P, NDC, JW], BF16, kind="ExternalInput").ap()
    wvb = nc.dram_tensor("wvb", [P, NDC, JW], BF16, kind="ExternalInput").ap()
    # wo split by key-chunk parity: even chunks feed lh=0, odd feed lh=1
    woE = nc.dram_tensor("woE", [P, NJC // 2, D], BF16, kind="ExternalInput").ap()
    woO = nc.dram_tensor("woO", [P, NJC // 2, D], BF16, kind="ExternalInput").ap()
    bqb = nc.dram_tensor("bqb", [P, HPC], F32, kind="ExternalInput").ap()
    bkb = nc.dram_tensor("bkb", [P, HPC], F32, kind="ExternalInput").ap()
    bob = nc.dram_tensor("bob", [P, NDC], F32, kind="ExternalInput").ap()
    # 4 diagonal-band mask patterns (1.0 = attend), [p][m][i]
    maskb = nc.dram_tensor("maskb", [P, 4, IT], BF16, kind="ExternalInput").ap()
    onesb = nc.dram_tensor("onesb", [P, P], BF16, kind="ExternalInput").ap()
    out = nc.dram_tensor("out", [P, NDC, TSL], BF16, kind="ExternalOutput").ap()

    with tile.TileContext(nc) as tc:
        with (
            tc.tile_pool(name="dram", bufs=1, space="DRAM") as dram,
            tc.tile_pool(name="const", bufs=1) as cpool,
            tc.tile_pool(name="persist", bufs=1) as ppool,
            tc.tile_pool(name="small", bufs=2) as small,
            tc.tile_pool(name="epool", bufs=2) as epool,
        ):
            # ---- persistent SBUF tiles ----
            qT_sb = {}
            kT_sb = {}
            for lh in range(HPC):
                for b in range(B):
                    qT_sb[(lh, b)] = ppool.tile([P, S], BF16, name=f"qT_{lh}_{b}")
                    kT_sb[(lh, b)] = ppool.tile([P, S], BF16, name=f"kT_{lh}_{b}")
            v_sb = {
                b: ppool.tile([P, NJC, JW], BF16, name=f"v_{b}") for b in range(B)
            }
            ya_sb = {
                lh: ppool.tile([P, NCORES, TSL], BF16, name=f"ya{lh}")
                for lh in range(HPC)
            }
            wo_sb = {
                0: ppool.tile([P, NJC // 2, D], BF16, name="woE"),
                1: ppool.tile([P, NJC // 2, D], BF16, name="woO"),
            }

            # per-local-head AllToAll buffers (blocks = dest core's i-slice)
            a2a_in = {
                lh: dram.tile([NCORES, DH, TSL], BF16, name=f"a2a_in_{lh}")
                for lh in range(HPC)
            }
            a2a_out = {
                lh: dram.tile([NCORES * DH, TSL], BF16, name=f"a2a_out_{lh}")
                for lh in range(HPC)
            }

            # tiny warmup AllToAll: absorbs the ~11 us CC arming cost
            # while the projections run, so the real AllToAll(0) starts fast
            a2aw_in = dram.tile([NCORES, P, 4], BF16, name="a2aw_in")
            a2aw_out = dram.tile([NCORES * P, 4], BF16, name="a2aw_out")

            # ---- constants / weights ----
            wq_w = cpool.tile([P, NDC, JW], BF16)
            wk_w = cpool.tile([P, NDC, JW], BF16)
            wv_w = cpool.tile([P, NDC, JW], BF16)
            bq_sb = cpool.tile([P, HPC], F32)
            bk_sb = cpool.tile([P, HPC], F32)
            bo_sb = cpool.tile([P, NDC], F32)
            mask_sb = cpool.tile([P, 4, IT], BF16)
            ones_sb = cpool.tile([P, P], BF16)

            # warmup AllToAll emission: DVE memset + gpsimd-queue DMAs so
            # the Sync queue (x/weight streaming) is untouched
            wsrc = cpool.tile([P, 4], BF16, name="wsrc")
            nc.vector.memset(wsrc[:], 1.0)
            for gq in range(NCORES):
                nc.gpsimd.dma_start(a2aw_in[gq, :, :], wsrc[:])
            nc.gpsimd.collective_compute(
                "AllToAll",
                ALU.bypass,
                replica_groups=[list(range(NCORES))],
                ins=[a2aw_in[:].opt()],
                outs=[a2aw_out[:].opt()],
            )

            # ---------- projections (SBUF-resident outputs) ----------
            def proj_pass(tag):
                """q/k projections for both heads + V, streamed over the
                8 token slices.  V chains (bufs=2) run VLAG dc ahead so the
                slice boundary never waits on the q/k psum drains."""
                with (
                    tc.tile_pool(name=f"xpool{tag}", bufs=1) as xpool,
                    tc.tile_pool(name=f"psum_{tag}", bufs=1, space="PSUM") as psp,
                ):
                    for ts in range(NTS):
                        b, lt0 = ts // NIT, (ts % NIT) * IT
                        xs = []
                        for g in range(NXS):
                            xg = xpool.tile(
                                [P, XSUB, IT],
                                BF16,
                                tag="x",
                                bufs=2,
                                name=f"x{tag}_{ts}_{g}",
                            )
                            if ts == 0 and g == 0:
                                nc.sync.dma_start(
                                    xg[:, 0:2, :], xb[ts, :, 0:2, :]
                                )
                                nc.sync.dma_start(
                                    xg[:, 2:XSUB, :], xb[ts, :, 2:XSUB, :]
                                )
                            else:
                                nc.sync.dma_start(
                                    xg[:], xb[ts, :, g * XSUB : (g + 1) * XSUB, :]
                                )
                            xs.append(xg)
                        if ts == 0:
                            # weights AFTER ts0's x tiles so x isn't queued
                            # behind 6 MB on the shared HBM bandwidth
                            nc.sync.dma_start(wv_w[:, 0:4, :], wvb[:, 0:4, :])
                            nc.sync.dma_start(wk_w[:, 0:2, :], wkb[:, 0:2, :])
                            nc.sync.dma_start(wq_w[:, 0:2, :], wqb[:, 0:2, :])
                            nc.sync.dma_start(wv_w[:, 4:8, :], wvb[:, 4:8, :])
                            nc.sync.dma_start(wk_w[:, 2:5, :], wkb[:, 2:5, :])
                            nc.sync.dma_start(wq_w[:, 2:5, :], wqb[:, 2:5, :])
                            nc.sync.dma_start(wk_w[:, 5:8, :], wkb[:, 5:8, :])
                            nc.sync.dma_start(wq_w[:, 5:8, :], wqb[:, 5:8, :])
                            nc.sync.dma_start(wk_w[:, 8:12, :], wkb[:, 8:12, :])
                            nc.sync.dma_start(wq_w[:, 8:12, :], wqb[:, 8:12, :])
                            nc.sync.dma_start(wk_w[:, 12:16, :], wkb[:, 12:16, :])
                            nc.sync.dma_start(wq_w[:, 12:16, :], wqb[:, 12:16, :])
                            nc.sync.dma_start(wv_w[:, 8:16, :], wvb[:, 8:16, :])
                            nc.sync.dma_start(bk_sb[:], bkb)
                            nc.sync.dma_start(bq_sb[:], bqb)
                            nc.sync.dma_start(bo_sb[:], bob)
                            nc.sync.dma_start(mask_sb[:], maskb)
                            nc.sync.dma_start(ones_sb[:], onesb)
                        if ts == 2:
                            nc.sync.dma_start(wo_sb[0][:], woE)
                        if ts == 4:
                            nc.sync.dma_start(wo_sb[1][:], woO)

                        def xchunk(dc):
                            return xs[dc // XSUB][:, dc % XSUB, :]

                        pqk = {
                            nm: psp.tile(
                                [P, IT], F32, tag=nm, name=f"p{nm}_{ts}"
                            )
                            for nm in ("k0", "k1", "q0", "q1")
                        }
                        pv = {
                            half: psp.tile(
                                [P, 2, JW],
                                F32,
                                tag=f"v{half}",
                                bufs=2,
                                name=f"pv{ts}{half}",
                            )
                            for half in range(2)
                        }

                        def emit_v(dc):
                            sp = dc == NDC - 1
                            for tc2 in range(IT // P):
                                nc.tensor.matmul(
                                    pv[tc2 // 2][:, tc2 % 2, :],
                                    lhsT=xchunk(dc)[:, tc2 * P : (tc2 + 1) * P],
                                    rhs=wv_w[:, dc, :],
                                    start=(dc == 0 and tc2 % 2 == 0),
                                    stop=sp,
                                    skip_group_check=True,
                                )

                        # V warmup: first VLAG dc of the V chains keep the
                        # PE busy while this slice's q/k psum banks drain
                        for dc in range(VLAG):
                            emit_v(dc)
                        for dc in range(NDC):
                            st, sp = dc == 0, dc == NDC - 1
                            for h in range(HPC):
                                nc.tensor.matmul(
                                    pqk[f"k{h}"][:],
                                    lhsT=wk_w[:, dc, h * DH : (h + 1) * DH],
                                    rhs=xchunk(dc),
                                    start=st,
                                    stop=sp,
                                )
                                nc.tensor.matmul(
                                    pqk[f"q{h}"][:],
                                    lhsT=wq_w[:, dc, h * DH : (h + 1) * DH],
                                    rhs=xchunk(dc),
                                    start=st,
                                    stop=sp,
                                )
                            if dc < NDC - VLAG:
                                emit_v(dc + VLAG)
                        # epilogues on DVE: bias add, write bf16 persistents
                        for h in range(HPC):
                            nc.vector.tensor_tensor(
                                kT_sb[(h, b)][:, lt0 : lt0 + IT],
                                pqk[f"k{h}"][:],
                                bk_sb[:, h : h + 1].to_broadcast([P, IT]),
                                ALU.add,
                            )
                            nc.vector.tensor_tensor(
                                qT_sb[(h, b)][:, lt0 : lt0 + IT],
                                pqk[f"q{h}"][:],
                                bq_sb[:, h : h + 1].to_broadcast([P, IT]),
                                ALU.add,
                            )
                        for half in range(2):
                            # v bias folded into bo on the host
                            nc.vector.tensor_copy(
                                v_sb[b][
                                    :,
                                    lt0 // P + 2 * half : lt0 // P + 2 * half + 2,
                                    :,
                                ],
                                pv[half][:],
                            )

            # ---------- attention for one local head + its AllToAll ----------
            def attention(lh, pre_cc=None):
                with (
                    tc.tile_pool(name=f"psS{lh}", bufs=2, space="PSUM") as psS,
                    tc.tile_pool(name=f"psO{lh}", bufs=2, space="PSUM") as psO,
                    tc.tile_pool(name=f"psR{lh}", bufs=2, space="PSUM") as psR,
                ):
                    # softmax denominators: exp groups accumulate pairwise
                    # into racc2 [P,2,IT] on the DVE (one op per group),
                    # contracted by TWO ones-matmuls per i-tile.  That
                    # contraction + epilogue are emitted after the next
                    # tile's first scores group so the PE never waits on
                    # the DVE tail.
                    pending = None

                    def flush_pending():
                        nonlocal pending
                        if pending is None:
                            return
                        racc2, po, pr, b, it, had_direct = pending
                        pending = None
                        if racc2 is not None:
                            nc.tensor.matmul(
                                pr[:],
                                lhsT=ones_sb[:],
                                rhs=racc2[:, 0, :],
                                start=not had_direct,
                                stop=False,
                            )
                            nc.tensor.matmul(
                                pr[:],
                                lhsT=ones_sb[:],
                                rhs=racc2[:, 1, :],
                                start=False,
                                stop=True,
                            )
                        rinv = small.tile(
                            [P, IT], F32, tag="rinv", name=f"ri{lh}{b}{it}"
                        )
                        nc.vector.reciprocal_approx_fast(rinv[:], pr[:])
                        # v-bias and output bias are folded into bo on the
                        # host (softmax rows sum to 1), so y is just po/r
                        y_sb = small.tile(
                            [P, IT], BF16, tag="y", bufs=8, name=f"y{lh}{b}{it}"
                        )
                        nc.vector.tensor_tensor(y_sb[:], po[:], rinv[:], ALU.mult)
                        g = NIT * b + it  # destination core / a2a block
                        nc.sync.dma_start(a2a_in[lh][g, :, :], y_sb[:])

                    for b in range(B):
                        kT = kT_sb[(lh, b)]
                        for it in range(NIT):
                            q_ap = qT_sb[(lh, b)][:, it * IT : (it + 1) * IT]
                            njc = (it + 1) * (IT // P)
                            ngr = njc // 2
                            po = psO.tile([P, IT], F32, tag="o", name=f"po{lh}{b}{it}")
                            pr = psR.tile([P, IT], F32, tag="r", name=f"pr{lh}{b}{it}")
                            # groups routed directly to the PE rowsum (no
                            # DVE): the last group; for it=0 both groups
                            d_all = b == B - 1 and it == NIT - 1
                            if d_all:
                                # the tile that gates this head's AllToAll:
                                # last TWO groups direct on the PE, so the
                                # fold's racc2 dependency completes 3 groups
                                # before the tile ends (short trigger tail)
                                direct = {ngr - 2, ngr - 1}
                            elif ngr == 2:
                                direct = set()
                            else:
                                direct = {ngr - 1}
                            racc2 = small.tile(
                                [P, 2, IT],
                                BF16,
                                tag="racc",
                                name=f"ra{lh}{b}{it}",
                            )
                            first_direct = [True]

                            def emit_av(e_tile, jg):
                                for k2 in range(2):
                                    jc = jg * 2 + k2
                                    nc.tensor.matmul(
                                        po[:],
                                        lhsT=v_sb[b][:, jc, lh * DH : (lh + 1) * DH],
                                        rhs=e_tile[:, k2, :],
                                        start=(jc == 0),
                                        stop=(jc == njc - 1),
                                    )
                                if jg in direct:
                                    # rowsum straight on the PE, riding the
                                    # AV pipeline so exp/mask are long done
                                    for k2 in range(2):
                                        nc.tensor.matmul(
                                            pr[:],
                                            lhsT=ones_sb[:],
                                            rhs=e_tile[:, k2, :],
                                            start=first_direct[0],
                                            stop=(
                                                racc2 is None
                                                and jg == ngr - 1
                                                and k2 == 1
                                            ),
                                        )
                                        first_direct[0] = False

                            pipe = []
                            e_diag = None
                            for jg in range(ngr):
                                ps2 = psS.tile([P, 2, IT], F32, tag="s")
                                for k2 in range(2):
                                    jc = jg * 2 + k2
                                    nc.tensor.matmul(
                                        ps2[:, k2, :],
                                        lhsT=kT[:, jc * P : (jc + 1) * P],
                                        rhs=q_ap,
                                        start=True,
                                        stop=True,
                                    )
                                if jg == 0:
                                    # prev i-tile's rowsum matmuls slot in
                                    # behind this tile's first scores
                                    flush_pending()
                                if jg >= ngr - 2:
                                    # the two diagonal groups share one tile
                                    # so ONE batched [P,4,IT] mask op covers
                                    # them both
                                    if e_diag is None:
                                        e_diag = epool.tile(
                                            [P, 4, IT], BF16, tag="ed",
                                            bufs=2, name=f"ed{lh}{b}{it}"
                                        )
                                    half = jg - (ngr - 2)
                                    e_sb = e_diag[:, 2 * half : 2 * half + 2, :]
                                else:
                                    e_sb = epool.tile(
                                        [P, 2, IT], BF16, tag="e",
                                        bufs=3, name=f"e{lh}{b}{it}{jg}"
                                    )[:]
                                nc.scalar.activation(
                                    e_sb, ps2[:], AF.Exp, scale=1.0 / SCALE
                                )
                                if jg == ngr - 1:
                                    nc.vector.tensor_tensor(
                                        e_diag[:], e_diag[:], mask_sb[:], ALU.mult
                                    )
                                # rowsum accumulation on the DVE: groups 0+1
                                # combine in one op; later non-direct groups
                                # add pairwise.  The second-to-last (masked)
                                # group's add is deferred to after the mask.
                                if racc2 is not None:
                                    if jg == 1 and ngr == 2:
                                        # both groups are in e_diag; combine
                                        # after the mask op (DVE-ordered)
                                        nc.vector.tensor_tensor(
                                            racc2[:],
                                            e_diag[:, 0:2, :],
                                            e_diag[:, 2:4, :],
                                            ALU.add,
                                        )
                                    elif jg == 1:
                                        nc.vector.tensor_tensor(
                                            racc2[:], pipe[0][0], e_sb, ALU.add
                                        )
                                    elif (
                                        jg == ngr - 1
                                        and ngr > 2
                                        and ngr - 2 not in direct
                                    ):
                                        nc.vector.tensor_tensor(
                                            racc2[:],
                                            racc2[:],
                                            e_diag[:, 0:2, :],
                                            ALU.add,
                                        )
                                    elif 1 < jg < ngr - 2:
                                        nc.vector.tensor_tensor(
                                            racc2[:], racc2[:], e_sb, ALU.add
                                        )
                                # AV lags two groups so the PE never waits
                                # on the exp/mask chain
                                pipe.append((e_sb, jg))
                                if len(pipe) > 2:
                                    emit_av(*pipe.pop(0))
                            for ent in pipe:
                                emit_av(*ent)
                            pending = (racc2, po, pr, b, it, bool(direct))
                    flush_pending()  # before this lh's collective
                    if pre_cc is not None:
                        pre_cc()
                nc.gpsimd.collective_compute(
                    "AllToAll",
                    ALU.bypass,
                    replica_groups=[list(range(NCORES))],
                    ins=[a2a_in[lh][:].opt()],
                    outs=[a2a_out[lh][:].opt()],
                )

            def ya_dma(lh):
                half = NCORES // 2
                ro = a2a_out[lh][:].rearrange("(s p) i -> p s i", p=P)
                nc.sync.dma_start(ya_sb[lh][:, :half, :], ro[:, :half, :])
                nc.sync.dma_start(ya_sb[lh][:, half:, :], ro[:, half:, :])

            proj_pass("a")
            attention(0)
            # ya_dma(0) is emitted AFTER att(1)'s a2a_in writes (pre_cc) so
            # the in-order Sync queue never blocks them on collective(0)
            attention(1, pre_cc=lambda: ya_dma(0))
            ya_dma(1)

            # ---------- output projection on own token slice ----------
            # ya_sb[lh] block s holds key chunk jc = 2s + lh, i.e. the s-th
            # chunk of wo_sb[lh] (parity-split).  ALL lh=0 matmuls run first
            # (partials staged to SBUF) so they cover the lh=1 AllToAll;
            # lh=1 matmuls then reuse the psum banks and the DVE combines
            # partial + psum + bias.  The two sub-chains are interleaved so
            # consecutive matmuls hit different psum banks.
            with (
                tc.tile_pool(name="opart", bufs=1) as opart,
                tc.tile_pool(name="ostage", bufs=2) as ostage,
                tc.tile_pool(name="psout", bufs=4, space="PSUM") as psout,
            ):
                EG = 2  # e-chunks per psum tile
                NEG = NDC // EG
                parts = [
                    opart.tile([P, EG, TSL], BF16, name=f"part{eg}")
                    for eg in range(NEG)
                ]

                def emit_mms(lh, eg, ps):
                    for s in range(NCORES):
                        for sub in range(EG):
                            ec = eg * EG + sub
                            nc.tensor.matmul(
                                ps[:, sub, :],
                                lhsT=wo_sb[lh][:, s, ec * P : ec * P + P],
                                rhs=ya_sb[lh][:, s, :],
                                start=(s == 0),
                                stop=(s == NCORES - 1),
                            )

                # pass 1: lh=0 into psum, drain raw partials to SBUF
                for eg in range(NEG):
                    ps = psout.tile([P, EG, TSL], F32, tag="out", name=f"p0_{eg}")
                    emit_mms(0, eg, ps)
                    nc.vector.tensor_copy(parts[eg][:], ps[:])
                # pass 2: lh=1 into psum, combine with partial + bias, store
                for eg in range(NEG):
                    ps = psout.tile([P, EG, TSL], F32, tag="out", name=f"p1_{eg}")
                    emit_mms(1, eg, ps)
                    ost = ostage.tile([P, EG, TSL], BF16, tag="ost", name=f"os{eg}")
                    nc.vector.tensor_tensor(ost[:], ps[:], parts[eg][:], ALU.add)
                    nc.vector.tensor_tensor(
                        ost[:],
                        ost[:],
                        bo_sb[:, eg * EG : (eg + 1) * EG, None].to_broadcast(
                            [P, EG, TSL]
                        ),
                        ALU.add,
                    )
                    nc.sync.dma_start(out[:, eg * EG : (eg + 1) * EG, :], ost[:])

    nc.finalize()  # bacc compile: regalloc etc. -- required before execution
    return nc


_PROGRAM = None


def _get_program():
    global _PROGRAM
    if _PROGRAM is None:
        _PROGRAM = build_program()
    return _PROGRAM


def _host_prep(x, mask, wq, bq, wk, bk, wv, bv, wo, bo):
    """Build the 8 per-core input maps (host-side marshalling only)."""
    import ml_dtypes

    f = np.float32
    bf = ml_dtypes.bfloat16
    x2 = np.asarray(x, dtype=f).reshape(T, D)
    # [ts][p][dc][t] blocked x^T so every DMA descriptor is contiguous
    xb = x2.T.reshape(NDC, P, NTS, IT).transpose(2, 1, 0, 3).astype(bf)

    # fold the v-bias through the output projection: softmax rows sum to 1
    # so attn@(v+bv) @ wo^T + bo == attn@v @ wo^T + (wo @ bv + bo)
    bo_eff = np.asarray(bo, dtype=f) + np.asarray(wo, dtype=f) @ np.asarray(bv, dtype=f)
    woT = np.asarray(wo, dtype=f).T.reshape(NJC, P, D)  # [jc][p][e]
    woE = woT[0::2].transpose(1, 0, 2).astype(bf)  # [p][s][e], jc = 2s
    woO = woT[1::2].transpose(1, 0, 2).astype(bf)  # [p][s][e], jc = 2s+1
    bo_b = np.ascontiguousarray(bo_eff.reshape(NDC, P).T)

    # diagonal-band mask patterns from the provided mask (True = masked out)
    mask_np = np.asarray(mask)
    maskp = np.empty((4, P, IT), dtype=f)
    for m in range(4):
        maskp[m] = (~mask_np[0:IT, m * P : (m + 1) * P]).T.astype(f)
    maskb = maskp.transpose(1, 0, 2).astype(bf)  # [p][m][i]

    wq_, wk_, wv_ = (np.asarray(w, dtype=f) for w in (wq, wk, wv))
    bq_, bk_ = (np.asarray(v_, dtype=f) for v_ in (bq, bk))

    in_maps = []
    for c in range(NCORES):
        j0, j1 = c * JW, (c + 1) * JW
        in_maps.append(
            {
                "xb": xb,
                "wqb": wq_[j0:j1, :].T.reshape(NDC, P, JW).transpose(1, 0, 2).astype(bf),
                "wkb": wk_[j0:j1, :].T.reshape(NDC, P, JW).transpose(1, 0, 2).astype(bf),
                "wvb": wv_[j0:j1, :].T.reshape(NDC, P, JW).transpose(1, 0, 2).astype(bf),
                "woE": woE,
                "woO": woO,
                "bqb": np.ascontiguousarray(bq_[j0:j1].reshape(HPC, P).T),
                "bkb": np.ascontiguousarray(bk_[j0:j1].reshape(HPC, P).T),
                "bob": bo_b,
                "maskb": maskb,
                "onesb": np.ones((P, P), dtype=bf),
            }
        )
    return in_maps


LAST_RESULTS = None  # BassKernelResults of the most recent run (for test.py)


def _assemble(per_core_outs):
    """[P, NDC, TSL] blocked slices -> full [B, S, D] output."""
    outT = np.concatenate(
        [
            np.asarray(o, dtype=np.float32)
            .reshape(P, NDC, TSL)
            .transpose(1, 0, 2)
            .reshape(D, TSL)
            for o in per_core_outs
        ],
        axis=1,
    )
    return np.ascontiguousarray(outT.T).reshape(B, S, D).astype(np.float32)


def kernel(x, mask, wq, bq, wk, bk, wv, bv, wo, bo):
    global LAST_RESULTS
    from concourse.bass_utils import run_bass_kernel_spmd

    nc = _get_program()
    in_maps = _host_prep(x, mask, wq, bq, wk, bk, wv, bv, wo, bo)
    trace = os.environ.get("KERNEL_TRACE", "") == "1"
    kwargs = {}
    if os.environ.get("KERNEL_TRACE_ALL", "") == "1":
        kwargs["trace_cores"] = list(range(NCORES))
        kwargs["stitch_traces"] = True
    res = run_bass_kernel_spmd(
        nc, in_maps, core_ids=list(range(NCORES)), trace=trace, **kwargs
    )
    LAST_RESULTS = res
    return _assemble([res.results[c]["out"] for c in range(NCORES)])


# revision 32
# speedup vs baseline: 1.0908x; 1.0078x over previous
"""Trainium2 Bass kernel for nn_MultiHeadAttention (B=2, S=2048, D=2048, H=16).

Sharding: tensor-parallel over heads -- each of the 8 cores owns 2 heads
(both batches) for the q/k/v projections and attention, then two 8-way
AllToAlls (one per local head) convert the head-sharded attention output
Y^T into a token-sharded layout, and each core computes a disjoint
512-token slice of the output projection (no all-reduce needed).

Key structure (evolved against neuron-profile traces):
- All matmul operands are bf16; psum accumulation stays f32.  The PE
  sustains ~0.515 ns/col in collective-bearing programs (1.94 GHz;
  CC-free microbenches run at 2.4 GHz -- collectives throttle the PE
  ~18% program-wide, and the SWDGE remote_dma path that would avoid
  them crashes this runtime).  ~710k matmul cols/core => ~366 us PE
  floor; everything else hides behind it.
- q^T / k^T / v live in SBUF between phases; host pre-blocks all DRAM
  inputs into the exact [partition][...] layouts (contiguous DMAs).
- Projection streams x once (x tiles DMA'd BEFORE the 6 MB of weights
  so they are not starved on HBM bandwidth); 8 interleaved psum chains;
  V chains are double-buffered and run VLAG dc ahead so slice
  boundaries never wait on the q/k psum drains.
- Attention epilogue balance (per head-batch: PE 25us, DVE ~24, ACT 17):
  exp groups accumulate pairwise into a [P,2,IT] bf16 racc2 on the DVE
  (groups 0+1 combine in one op; the last group contracts directly on
  the PE, riding the AV pipeline); the two diagonal groups share one
  [P,4,IT] tile so a single batched DVE op applies the causal mask; AV
  emission lags two groups so the PE never waits on the exp/mask chain.
  V-bias and output bias fold into bo_eff = bo + wo@bv on the host
  (softmax rows sum to 1).
- A tiny warmup AllToAll during the projections absorbs the ~11 us CC
  arming cost so AllToAll(0) starts ~1 us after its trigger.
- Phase order: proj -> att(lh0) -> AllToAll(0) -> att(lh1) -> ya0
  readback -> AllToAll(1) -> ya1 readback -> out-proj.  att(lh1) covers
  AllToAll(0); out-proj pass 1 (all lh0 matmuls, partials staged in
  SBUF) covers AllToAll(1).  ya_dma(0) is emitted AFTER att(lh1)'s
  a2a_in writes so the in-order Sync queue never blocks them (the
  baseline lost 23 us + an out-proj DVFS ramp to that).
- Out-proj interleaves its two psum sub-chains (LDWEIGHTS stays hidden)
  and stages the output in bf16 (host upcasts) to halve the final DMA.
"""

import os
import sys

import numpy as np

_REPO = "/opt/trn_rl_repo"
if _REPO not in sys.path:
    sys.path.insert(0, _REPO)

from concourse import bacc, mybir, tile  # noqa: E402
import concourse.bass as bass  # noqa: E402

B, S, D, H = 2, 2048, 2048, 16
DH = D // H  # 128
NCORES = 8
HPC = H // NCORES  # heads per core = 2
JW = HPC * DH  # per-core head-feature width = 256
T = B * S  # 4096 flattened tokens
TSL = T // NCORES  # per-core output token slice = 512
SCALE = float(np.sqrt(DH))

F32 = mybir.dt.float32
BF16 = mybir.dt.bfloat16
AF = mybir.ActivationFunctionType
ALU = bass.mybir.AluOpType

P = 128
IT = 512  # query i-tile width
NIT = S // IT  # 4 i-tiles per (batch, head)
NJC = S // P  # 16 key chunks per batch
NDC = D // P  # 16 contraction chunks
NTS = T // IT  # 8 token slices (batch 0 first, then batch 1)
XSUB = 8  # x dc-chunks per sub-tile (few big DMAs win)
NXS = NDC // XSUB  # 2 x sub-tiles per slice
VLAG = 6  # V chains run this many dc ahead (slice-boundary warmup)


def build_program():
    nc = bacc.Bacc(
        "TRN2",
        target_bir_lowering=False,
        debug=False,
        num_devices=NCORES,
    )

    # ---- kernel I/O (host pre-blocked; per-core values via in_maps) ----
    xb = nc.dram_tensor("xb", [NTS, P, NDC, IT], BF16, kind="ExternalInput").ap()
    wqb = nc.dram_tensor("wqb", [P, NDC, JW], BF16, kind="ExternalInput").ap()
    wkb = nc.dram_tensor("wkb", [P, NDC, JW], BF16, kind="ExternalInput").ap()
    wvb = nc.dram_tensor("wvb", [P, NDC, JW], BF16, kind="ExternalInput").ap()
    # wo split by key-chunk parity: even chunks feed lh=0, odd feed lh=1
    woE = nc.dram_tensor("woE", [P, NJC // 2, D], BF16, kind="ExternalInput").ap()
    woO = nc.dram_tensor("woO", [P, NJC // 2, D], BF16, kind="ExternalInput").ap()
    bqb = nc.dram_tensor("bqb", [P, HPC], F32, kind="ExternalInput").ap()
    bkb = nc.dram_tensor("bkb", [P, HPC], F32, kind="ExternalInput").ap()
    bob = nc.dram_tensor("bob", [P, NDC], F32, kind="ExternalInput").ap()
    # 4 diagonal-band mask patterns (1.0 = attend), [p][m][i]
    maskb = nc.dram_tensor("maskb", [P, 4, IT], BF16, kind="ExternalInput").ap()
    onesb = nc.dram_tensor("onesb", [P, P], BF16, kind="ExternalInput").ap()
    out = nc.dram_tensor("out", [P, NDC, TSL], BF16, kind="ExternalOutput").ap()

    with tile.TileContext(nc) as tc:
        with (
            tc.tile_pool(name="dram", bufs=1, space="DRAM") as dram,
            tc.tile_pool(name="const", bufs=1) as cpool,
            tc.tile_pool(name="persist", bufs=1) as ppool,
            tc.tile_pool(name="small", bufs=2) as small,
            tc.tile_pool(name="epool", bufs=2) as epool,
        ):
            # ---- persistent SBUF tiles ----
            qT_sb = {}
            kT_sb = {}
            for lh in range(HPC):
                for b in range(B):
                    qT_sb[(lh, b)] = ppool.tile([P, S], BF16, name=f"qT_{lh}_{b}")
                    kT_sb[(lh, b)] = ppool.tile([P, S], BF16, name=f"kT_{lh}_{b}")
            v_sb = {
                b: ppool.tile([P, NJC, JW], BF16, name=f"v_{b}") for b in range(B)
            }
            ya_sb = {
                lh: ppool.tile([P, NCORES, TSL], BF16, name=f"ya{lh}")
                for lh in range(HPC)
            }
            wo_sb = {
                0: ppool.tile([P, NJC // 2, D], BF16, name="woE"),
                1: ppool.tile([P, NJC // 2, D], BF16, name="woO"),
            }

            # per-local-head AllToAll buffers (blocks = dest core's i-slice)
            a2a_in = {
                lh: dram.tile([NCORES, DH, TSL], BF16, name=f"a2a_in_{lh}")
                for lh in range(HPC)
            }
            a2a_out = {
                lh: dram.tile([NCORES * DH, TSL], BF16, name=f"a2a_out_{lh}")
                for lh in range(HPC)
            }

            # tiny warmup AllToAll: absorbs the ~11 us CC arming cost
            # while the projections run, so the real AllToAll(0) starts fast
            a2aw_in = dram.tile([NCORES, P, 4], BF16, name="a2aw_in")
            a2aw_out = dram.tile([NCORES * P, 4], BF16, name="a2aw_out")

            # ---- constants / weights ----
            wq_w = cpool.tile([P, NDC, JW], BF16)
            wk_w = cpool.tile([P, NDC, JW], BF16)
            wv_w = cpool.tile([P, NDC, JW], BF16)
            bq_sb = cpool.tile([P, HPC], F32)
            bk_sb = cpool.tile([P, HPC], F32)
            bo_sb = cpool.tile([P, NDC], F32)
            mask_sb = cpool.tile([P, 4, IT], BF16)
            ones_sb = cpool.tile([P, P], BF16)

            # warmup AllToAll emission: DVE memset + gpsimd-queue DMAs so
            # the Sync queue (x/weight streaming) is untouched
            wsrc = cpool.tile([P, 4], BF16, name="wsrc")
            nc.vector.memset(wsrc[:], 1.0)
            for gq in range(NCORES):
                nc.gpsimd.dma_start(a2aw_in[gq, :, :], wsrc[:])
            nc.gpsimd.collective_compute(
                "AllToAll",
                ALU.bypass,
                replica_groups=[list(range(NCORES))],
                ins=[a2aw_in[:].opt()],
                outs=[a2aw_out[:].opt()],
            )

            # ---------- projections (SBUF-resident outputs) ----------
            def proj_pass(tag):
                """q/k projections for both heads + V, streamed over the
                8 token slices.  V chains (bufs=2) run VLAG dc ahead so the
                slice boundary never waits on the q/k psum drains."""
                with (
                    tc.tile_pool(name=f"xpool{tag}", bufs=1) as xpool,
                    tc.tile_pool(name=f"psum_{tag}", bufs=1, space="PSUM") as psp,
                ):
                    for ts in range(NTS):
                        b, lt0 = ts // NIT, (ts % NIT) * IT
                        xs = []
                        for g in range(NXS):
                            xg = xpool.tile(
                                [P, XSUB, IT],
                                BF16,
                                tag="x",
                                bufs=2,
                                name=f"x{tag}_{ts}_{g}",
                            )
                            if ts == 0 and g == 0:
                                nc.sync.dma_start(
                                    xg[:, 0:2, :], xb[ts, :, 0:2, :]
                                )
                                nc.sync.dma_start(
                                    xg[:, 2:XSUB, :], xb[ts, :, 2:XSUB, :]
                                )
                            else:
                                nc.sync.dma_start(
                                    xg[:], xb[ts, :, g * XSUB : (g + 1) * XSUB, :]
                                )
                            xs.append(xg)
                        if ts == 0:
                            # weights AFTER ts0's x tiles so x isn't queued
                            # behind 6 MB on the shared HBM bandwidth
                            nc.sync.dma_start(wv_w[:, 0:4, :], wvb[:, 0:4, :])
                            nc.sync.dma_start(wk_w[:, 0:2, :], wkb[:, 0:2, :])
                            nc.sync.dma_start(wq_w[:, 0:2, :], wqb[:, 0:2, :])
                            nc.sync.dma_start(wv_w[:, 4:8, :], wvb[:, 4:8, :])
                            nc.sync.dma_start(wk_w[:, 2:5, :], wkb[:, 2:5, :])
                            nc.sync.dma_start(wq_w[:, 2:5, :], wqb[:, 2:5, :])
                            nc.sync.dma_start(wk_w[:, 5:8, :], wkb[:, 5:8, :])
                            nc.sync.dma_start(wq_w[:, 5:8, :], wqb[:, 5:8, :])
                            nc.sync.dma_start(wk_w[:, 8:12, :], wkb[:, 8:12, :])
                            nc.sync.dma_start(wq_w[:, 8:12, :], wqb[:, 8:12, :])
                            nc.sync.dma_start(wk_w[:, 12:16, :], wkb[:, 12:16, :])
                            nc.sync.dma_start(wq_w[:, 12:16, :], wqb[:, 12:16, :])
                            nc.sync.dma_start(wv_w[:, 8:16, :], wvb[:, 8:16, :])
                            nc.sync.dma_start(bk_sb[:], bkb)
                            nc.sync.dma_start(bq_sb[:], bqb)
                            nc.sync.dma_start(bo_sb[:], bob)
                            nc.sync.dma_start(mask_sb[:], maskb)
                            nc.sync.dma_start(ones_sb[:], onesb)
                        if ts == 2:
                            nc.sync.dma_start(wo_sb[0][:], woE)
                        if ts == 4:
                            nc.sync.dma_start(wo_sb[1][:], woO)

                        def xchunk(dc):
                            return xs[dc // XSUB][:, dc % XSUB, :]

                        pqk = {
                            nm: psp.tile(
                                [P, IT], F32, tag=nm, name=f"p{nm}_{ts}"
                            )
                            for nm in ("k0", "k1", "q0", "q1")
                        }
                        pv = {
                            half: psp.tile(
                                [P, 2, JW],
                                F32,
                                tag=f"v{half}",
                                bufs=2,
                                name=f"pv{ts}{half}",
                            )
                            for half in range(2)
                        }

                        def emit_v(dc):
                            sp = dc == NDC - 1
                            for tc2 in range(IT // P):
                                nc.tensor.matmul(
                                    pv[tc2 // 2][:, tc2 % 2, :],
                                    lhsT=xchunk(dc)[:, tc2 * P : (tc2 + 1) * P],
                                    rhs=wv_w[:, dc, :],
                                    start=(dc == 0 and tc2 % 2 == 0),
                                    stop=sp,
                                    skip_group_check=True,
                                )

                        # V warmup: first VLAG dc of the V chains keep the
                        # PE busy while this slice's q/k psum banks drain
                        for dc in range(VLAG):
                            emit_v(dc)
                        for dc in range(NDC):
                            st, sp = dc == 0, dc == NDC - 1
                            for h in range(HPC):
                                nc.tensor.matmul(
                                    pqk[f"k{h}"][:],
                                    lhsT=wk_w[:, dc, h * DH : (h + 1) * DH],
                                    rhs=xchunk(dc),
                                    start=st,
                                    stop=sp,
                                )
                                nc.tensor.matmul(
                                    pqk[f"q{h}"][:],
                                    lhsT=wq_w[:, dc, h * DH : (h + 1) * DH],
                                    rhs=xchunk(dc),
                                    start=st,
                                    stop=sp,
                                )
                            if dc < NDC - VLAG:
                                emit_v(dc + VLAG)
                        # epilogues on DVE: bias add, write bf16 persistents
                        for h in range(HPC):
                            nc.vector.tensor_tensor(
                                kT_sb[(h, b)][:, lt0 : lt0 + IT],
                                pqk[f"k{h}"][:],
                                bk_sb[:, h : h + 1].to_broadcast([P, IT]),
                                ALU.add,
                            )
                            nc.vector.tensor_tensor(
                                qT_sb[(h, b)][:, lt0 : lt0 + IT],
                                pqk[f"q{h}"][:],
                                bq_sb[:, h : h + 1].to_broadcast([P, IT]),
                                ALU.add,
                            )
                        for half in range(2):
                            # v bias folded into bo on the host
                            nc.vector.tensor_copy(
                                v_sb[b][
                                    :,
                                    lt0 // P + 2 * half : lt0 // P + 2 * half + 2,
                                    :,
                                ],
                                pv[half][:],
                            )

            # ---------- attention for one local head + its AllToAll ----------
            def attention(lh, pre_cc=None):
                with (
                    tc.tile_pool(name=f"psS{lh}", bufs=2, space="PSUM") as psS,
                    tc.tile_pool(name=f"psO{lh}", bufs=2, space="PSUM") as psO,
                    tc.tile_pool(name=f"psR{lh}", bufs=2, space="PSUM") as psR,
                ):
                    # softmax denominators: exp groups accumulate pairwise
                    # into racc2 [P,2,IT] on the DVE (one op per group),
                    # contracted by TWO ones-matmuls per i-tile.  That
                    # contraction + epilogue are emitted after the next
                    # tile's first scores group so the PE never waits on
                    # the DVE tail.
                    pending = None

                    def flush_pending():
                        nonlocal pending
                        if pending is None:
                            return
                        racc2, po, pr, b, it, had_direct = pending
                        pending = None
                        if racc2 is not None:
                            nc.tensor.matmul(
                                pr[:],
                                lhsT=ones_sb[:],
                                rhs=racc2[:, 0, :],
                                start=not had_direct,
                                stop=False,
                            )
                            nc.tensor.matmul(
                                pr[:],
                                lhsT=ones_sb[:],
                                rhs=racc2[:, 1, :],
                                start=False,
                                stop=True,
                            )
                        rinv = small.tile(
                            [P, IT], F32, tag="rinv", name=f"ri{lh}{b}{it}"
                        )
                        nc.vector.reciprocal_approx_fast(rinv[:], pr[:])
                        # v-bias and output bias are folded into bo on the
                        # host (softmax rows sum to 1), so y is just po/r
                        y_sb = small.tile(
                            [P, IT], BF16, tag="y", bufs=8, name=f"y{lh}{b}{it}"
                        )
                        nc.vector.tensor_tensor(y_sb[:], po[:], rinv[:], ALU.mult)
                        g = NIT * b + it  # destination core / a2a block
                        nc.sync.dma_start(a2a_in[lh][g, :, :], y_sb[:])

                    for b in range(B):
                        kT = kT_sb[(lh, b)]
                        for it in range(NIT):
                            q_ap = qT_sb[(lh, b)][:, it * IT : (it + 1) * IT]
                            njc = (it + 1) * (IT // P)
                            ngr = njc // 2
                            po = psO.tile([P, IT], F32, tag="o", name=f"po{lh}{b}{it}")
                            pr = psR.tile([P, IT], F32, tag="r", name=f"pr{lh}{b}{it}")
                            # groups routed directly to the PE rowsum (no
                            # DVE): the last group; for it=0 both groups
                            d_all = b == B - 1 and it == NIT - 1
                            if d_all:
                                # the tile that gates this head's AllToAll:
                                # last TWO groups direct on the PE, so the
                                # fold's racc2 dependency completes 3 groups
                                # before the tile ends (short trigger tail)
                                direct = {ngr - 2, ngr - 1}
                            elif ngr == 2:
                                direct = set()
                            else:
                                direct = {ngr - 1}
                            racc2 = small.tile(
                                [P, 2, IT],
                                BF16,
                                tag="racc",
                                name=f"ra{lh}{b}{it}",
                            )
                            first_direct = [True]

                            def emit_av(e_tile, jg):
                                for k2 in range(2):
                                    jc = jg * 2 + k2
                                    nc.tensor.matmul(
                                        po[:],
                                        lhsT=v_sb[b][:, jc, lh * DH : (lh + 1) * DH],
                                        rhs=e_tile[:, k2, :],
                                        start=(jc == 0),
                                        stop=(jc == njc - 1),
                                    )
                                if jg in direct:
                                    # rowsum straight on the PE, riding the
                                    # AV pipeline so exp/mask are long done
                                    for k2 in range(2):
                                        nc.tensor.matmul(
                                            pr[:],
                                            lhsT=ones_sb[:],
                                            rhs=e_tile[:, k2, :],
                                            start=first_direct[0],
                                            stop=(
                                                racc2 is None
                                                and jg == ngr - 1
                                                and k2 == 1
                                            ),
                                        )
                                        first_direct[0] = False

                            pipe = []
                            e_diag = None
                            for jg in range(ngr):
                                ps2 = psS.tile([P, 2, IT], F32, tag="s")
                                for k2 in range(2):
                                    jc = jg * 2 + k2
                                    nc.tensor.matmul(
                                        ps2[:, k2, :],
                                        lhsT=kT[:, jc * P : (jc + 1) * P],
                                        rhs=q_ap,
                                        start=True,
                                        stop=True,
                                    )
                                if jg == 0:
                                    # prev i-tile's rowsum matmuls slot in
                                    # behind this tile's first scores
                                    flush_pending()
                                if jg >= ngr - 2:
                                    # the two diagonal groups share one tile
                                    # so ONE batched [P,4,IT] mask op covers
                                    # them both
                                    if e_diag is None:
                                        e_diag = epool.tile(
                                            [P, 4, IT], BF16, tag="ed",
                                            bufs=2, name=f"ed{lh}{b}{it}"
                                        )
                                    half = jg - (ngr - 2)
                                    e_sb = e_diag[:, 2 * half : 2 * half + 2, :]
                                else:
                                    e_sb = epool.tile(
                                        [P, 2, IT], BF16, tag="e",
                                        bufs=3, name=f"e{lh}{b}{it}{jg}"
                                    )[:]
                                nc.scalar.activation(
                                    e_sb, ps2[:], AF.Exp, scale=1.0 / SCALE
                                )
                                if jg == ngr - 1:
                                    nc.vector.tensor_tensor(
                                        e_diag[:], e_diag[:], mask_sb[:], ALU.mult
                                    )
                                # rowsum accumulation on the DVE: groups 0+1
                                # combine in one op; later non-direct groups
                                # add pairwise.  The second-to-last (masked)
                                # group's add is deferred to after the mask.
                                if racc2 is not None:
                                    if jg == 1 and ngr == 2:
                                        # both groups are in e_diag; combine
                                        # after the mask op (DVE-ordered)
                                        nc.vector.tensor_tensor(
                                            racc2[:],
                                            e_diag[:, 0:2, :],
                                            e_diag[:, 2:4, :],
                                            ALU.add,
                                        )
                                    elif jg == 1:
                                        nc.vector.tensor_tensor(
                                            racc2[:], pipe[0][0], e_sb, ALU.add
                                        )
                                    elif (
                                        jg == ngr - 1
                                        and ngr > 2
                                        and ngr - 2 not in direct
                                    ):
                                        nc.vector.tensor_tensor(
                                            racc2[:],
                                            racc2[:],
                                            e_diag[:, 0:2, :],
                                            ALU.add,
                                        )
                                    elif 1 < jg < ngr - 2:
                                        nc.vector.tensor_tensor(
                                            racc2[:], racc2[:], e_sb, ALU.add
                                        )
                                # AV lags two groups so the PE never waits
                                # on the exp/mask chain
                                pipe.append((e_sb, jg))
                                if len(pipe) > 2:
                                    emit_av(*pipe.pop(0))
                            for ent in pipe:
                                emit_av(*ent)
                            pending = (racc2, po, pr, b, it, bool(direct))
                    flush_pending()  # before this lh's collective
                    if pre_cc is not None:
                        pre_cc()
                nc.gpsimd.collective_compute(
                    "AllToAll",
                    ALU.bypass,
                    replica_groups=[list(range(NCORES))],
                    ins=[a2a_in[lh][:].opt()],
                    outs=[a2a_out[lh][:].opt()],
                )

            def ya_dma(lh):
                half = NCORES // 2
                ro = a2a_out[lh][:].rearrange("(s p) i -> p s i", p=P)
                nc.sync.dma_start(ya_sb[lh][:, :half, :], ro[:, :half, :])
                nc.sync.dma_start(ya_sb[lh][:, half:, :], ro[:, half:, :])

            proj_pass("a")
            attention(0)
            # ya_dma(0) is emitted AFTER att(1)'s a2a_in writes (pre_cc) so
            # the in-order Sync queue never blocks them on collective(0)
            attention(1, pre_cc=lambda: ya_dma(0))
            ya_dma(1)

            # ---------- output projection on own token slice ----------
            # ya_sb[lh] block s holds key chunk jc = 2s + lh, i.e. the s-th
            # chunk of wo_sb[lh] (parity-split).  ALL lh=0 matmuls run first
            # (partials staged to SBUF) so they cover the lh=1 AllToAll;
            # lh=1 matmuls then reuse the psum banks and the DVE combines
            # partial + psum + bias.  The two sub-chains are interleaved so
            # consecutive matmuls hit different psum banks.
            with (
                tc.tile_pool(name="opart", bufs=1) as opart,
                tc.tile_pool(name="ostage", bufs=2) as ostage,
                tc.tile_pool(name="psout", bufs=4, space="PSUM") as psout,
            ):
                EG = 2  # e-chunks per psum tile
                NEG = NDC // EG
                parts = [
                    opart.tile([P, EG, TSL], BF16, name=f"part{eg}")
                    for eg in range(NEG)
                ]

                def emit_mms(lh, eg, ps):
                    for s in range(NCORES):
                        for sub in range(EG):
                            ec = eg * EG + sub
                            nc.tensor.matmul(
                                ps[:, sub, :],
                                lhsT=wo_sb[lh][:, s, ec * P : ec * P + P],
                                rhs=ya_sb[lh][:, s, :],
                                start=(s == 0),
                                stop=(s == NCORES - 1),
                            )

                # pass 1: lh=0 into psum, drain raw partials to SBUF
                for eg in range(NEG):
                    ps = psout.tile([P, EG, TSL], F32, tag="out", name=f"p0_{eg}")
                    emit_mms(0, eg, ps)
                    nc.vector.tensor_copy(parts[eg][:], ps[:])
                # pass 2: lh=1 into psum, combine with partial + bias, store
                for eg in range(NEG):
                    ps = psout.tile([P, EG, TSL], F32, tag="out", name=f"p1_{eg}")
                    emit_mms(1, eg, ps)
                    ost = ostage.tile([P, EG, TSL], BF16, tag="ost", name=f"os{eg}")
                    nc.vector.tensor_tensor(ost[:], ps[:], parts[eg][:], ALU.add)
                    nc.vector.tensor_tensor(
                        ost[:],
                        ost[:],
                        bo_sb[:, eg * EG : (eg + 1) * EG, None].to_broadcast(
                            [P, EG, TSL]
                        ),
                        ALU.add,
                    )
                    nc.sync.dma_start(out[:, eg * EG : (eg + 1) * EG, :], ost[:])

    nc.finalize()  # bacc compile: regalloc etc. -- required before execution
    return nc


_PROGRAM = None


def _get_program():
    global _PROGRAM
    if _PROGRAM is None:
        _PROGRAM = build_program()
    return _PROGRAM


def _host_prep(x, mask, wq, bq, wk, bk, wv, bv, wo, bo):
    """Build the 8 per-core input maps (host-side marshalling only)."""
    import ml_dtypes

    f = np.float32
    bf = ml_dtypes.bfloat16
    x2 = np.asarray(x, dtype=f).reshape(T, D)
    # [ts][p][dc][t] blocked x^T so every DMA descriptor is contiguous
    xb = x2.T.reshape(NDC, P, NTS, IT).transpose(2, 1, 0, 3).astype(bf)

    # fold the v-bias through the output projection: softmax rows sum to 1
    # so attn@(v+bv) @ wo^T + bo == attn@v @ wo^T + (wo @ bv + bo)
    bo_eff = np.asarray(bo, dtype=f) + np.asarray(wo, dtype=f) @ np.asarray(bv, dtype=f)
    woT = np.asarray(wo, dtype=f).T.reshape(NJC, P, D)  # [jc][p][e]
    woE = woT[0::2].transpose(1, 0, 2).astype(bf)  # [p][s][e], jc = 2s
    woO = woT[1::2].transpose(1, 0, 2).astype(bf)  # [p][s][e], jc = 2s+1
    bo_b = np.ascontiguousarray(bo_eff.reshape(NDC, P).T)

    # diagonal-band mask patterns from the provided mask (True = masked out)
    mask_np = np.asarray(mask)
    maskp = np.empty((4, P, IT), dtype=f)
    for m in range(4):
        maskp[m] = (~mask_np[0:IT, m * P : (m + 1) * P]).T.astype(f)
    maskb = maskp.transpose(1, 0, 2).astype(bf)  # [p][m][i]

    wq_, wk_, wv_ = (np.asarray(w, dtype=f) for w in (wq, wk, wv))
    bq_, bk_ = (np.asarray(v_, dtype=f) for v_ in (bq, bk))

    in_maps = []
    for c in range(NCORES):
        j0, j1 = c * JW, (c + 1) * JW
        in_maps.append(
            {
                "xb": xb,
                "wqb": wq_[j0:j1, :].T.reshape(NDC, P, JW).transpose(1, 0, 2).astype(bf),
                "wkb": wk_[j0:j1, :].T.reshape(NDC, P, JW).transpose(1, 0, 2).astype(bf),
                "wvb": wv_[j0:j1, :].T.reshape(NDC, P, JW).transpose(1, 0, 2).astype(bf),
                "woE": woE,
                "woO": woO,
                "bqb": np.ascontiguousarray(bq_[j0:j1].reshape(HPC, P).T),
                "bkb": np.ascontiguousarray(bk_[j0:j1].reshape(HPC, P).T),
                "bob": bo_b,
                "maskb": maskb,
                "onesb": np.ones((P, P), dtype=bf),
            }
        )
    return in_maps


LAST_RESULTS = None  # BassKernelResults of the most recent run (for test.py)


def _assemble(per_core_outs):
    """[P, NDC, TSL] blocked slices -> full [B, S, D] output."""
    outT = np.concatenate(
        [
            np.asarray(o, dtype=np.float32)
            .reshape(P, NDC, TSL)
            .transpose(1, 0, 2)
            .reshape(D, TSL)
            for o in per_core_outs
        ],
        axis=1,
    )
    return np.ascontiguousarray(outT.T).reshape(B, S, D).astype(np.float32)


def kernel(x, mask, wq, bq, wk, bk, wv, bv, wo, bo):
    global LAST_RESULTS
    from concourse.bass_utils import run_bass_kernel_spmd

    nc = _get_program()
    in_maps = _host_prep(x, mask, wq, bq, wk, bk, wv, bv, wo, bo)
    trace = os.environ.get("KERNEL_TRACE", "") == "1"
    kwargs = {}
    if os.environ.get("KERNEL_TRACE_ALL", "") == "1":
        kwargs["trace_cores"] = list(range(NCORES))
        kwargs["stitch_traces"] = True
    res = run_bass_kernel_spmd(
        nc, in_maps, core_ids=list(range(NCORES)), trace=trace, **kwargs
    )
    LAST_RESULTS = res
    return _assemble([res.results[c]["out"] for c in range(NCORES)])
